# revision 1
# baseline (speedup 1.0000x reference)
"""BioRNN Trainium2 kernel (dev module).

Per-core math (batch-sharded 8-way, B=8 per core):
    z'_t = 0.2*(x_t @ w_in + noise_t + b_rec)        (precomputed, fp16, T layout)
    h_t  = 0.8*h_{t-1} + relu(z'_t + h_{t-1} @ W')   W' = 0.2*w_eff, fp16
Output h_t (B, T, 512) f32.

T layout: partition dim = n_rec slice (4 chunks of 128); free dim packs
(t, m, b): per-step supertile of 32 cols = 4 m-chunks x 8 batch.
  zbuf  sbuf fp16 (128, T*32)   col = t*32 + m*8 + b
  h16   sbuf fp16 (128, U*32)   ring of U steps, same col layout
  w16   sbuf fp16 (128, 4*512)  [p, k*512 + m*128 + c] = W'[k*128+p, m*128+c]
  xT16  sbuf fp16 (128, T*8)    col = t*8 + b   (n_in on partitions)

Recurrence step: 16 matmuls (lhsT = w16 tile (128,128), rhs = h16 slice
(128,8)) accumulate into psum (128, 2048) f32, bank m cols [512m, 512m+8).
Then per half (m pair): TT-add psum+zbuf -> r32, relu in place, STT
h_new = 0.8*h_old + r32 -> h16 ring (fp16).

Output path: PE-transpose h16 (128 r, blk t) -> psum fp16 (t, r), DVE cast
to f32 staging, DMA to out[b, t, r].
"""

import numpy as np
from contextlib import ExitStack

import concourse.bass as bass
import concourse.mybir as mybir
import concourse.tile as tile
from concourse import bacc
from concourse import dve_ops
from concourse.dve_spec import Spec, Src0, Src1, C0, relu as _dve_relu_expr, lower
from concourse.dve_uop import DveOpSpec
from concourse.masks import make_identity


def _register_relu_add_sc():
    """Register fused out = relu((in0 + in1) * s0) custom DVE op (idempotent)."""
    name = "RELU_ADD_SC_BIO"
    for o in dve_ops.OPS:
        if o.name == name:
            return o
    opcode = max(dve_ops._SUB_OPCODE_FOR_NAME.values()) + 1
    assert opcode < 0x20
    dve_ops._SUB_OPCODE_FOR_NAME[name] = opcode

    def _ref(in0, in1, c0, c1, c2):
        a = in0.astype(np.float32).reshape(in0.shape[0], -1)
        b = in1.astype(np.float32).reshape(in1.shape[0], -1)
        s = np.maximum(np.nan_to_num((a + b) * c0, nan=0.0, posinf=np.inf,
                                     neginf=-np.inf), 0)
        return s.reshape(in0.shape)

    spec = Spec(body=_dve_relu_expr((Src0 + Src1) * C0), reference=_ref)
    shas = {}
    for ver in ("v3", "v4"):
        s = DveOpSpec(name=name, opcode=opcode, uops=lower(spec, ver=ver),
                      rd1_en=True)
        shas[ver] = s.sha(ver)
    op = dve_ops.DveOp(name, spec, subdim=False, uops_sha=shas)
    dve_ops.OPS.append(op)
    dve_ops.CUSTOM_DVE_SPECS[name] = spec
    return op


RELU_ADD_SC = _register_relu_add_sc()

F32 = mybir.dt.float32
F16 = mybir.dt.float16
AOP = mybir.AluOpType

B = 8            # batch per core
R = 512          # n_rec
NIN = 128        # n_in
RC = 4           # r chunks (m and k)
SUP = RC * B     # 32 cols per step supertile
N_CORES = 8
ALPHA = 0.2
LEAK = 1.0 - ALPHA


def build_nc(T=1000, U=256, use_bacc=True):
    """Build the per-core Bass program. U = h-ring steps."""
    OBLK = 128  # output transpose block (steps)
    assert U % (2 * OBLK) == 0
    nc = bacc.Bacc() if use_bacc else bass.Bass()

    x_d = nc.dram_tensor("x_c", [B, T, NIN], F32, kind="ExternalInput").ap()
    n_d = nc.dram_tensor("noise_c", [B, T, R], F32, kind="ExternalInput").ap()
    w_d = nc.dram_tensor("w16", [R, R], F16, kind="ExternalInput").ap()
    wi_d = nc.dram_tensor("win16", [NIN, R], F16, kind="ExternalInput").ap()
    b_d = nc.dram_tensor("b32", [R], F32, kind="ExternalInput").ap()
    o_d = nc.dram_tensor("out_c", [B, T, R], F32, kind="ExternalOutput").ap()

    ZB = 64  # zmm steps per matmul (64 steps = 512 moving cols)

    with tile.TileContext(nc) as tc, ExitStack() as ctx:
        const = ctx.enter_context(tc.tile_pool(name="const", bufs=1))
        big = ctx.enter_context(tc.tile_pool(name="big", bufs=1))
        dram = ctx.enter_context(tc.tile_pool(name="dram", bufs=1, space="DRAM"))

        # ---- constants ----
        ident16 = const.tile([128, 128], F16)
        make_identity(nc, ident16[:, :])

        w16 = const.tile([128, RC * R], F16)
        nc.sync.dma_start(
            out=w16[:, :].rearrange("p (k m) -> p k m", m=R),
            in_=w_d.rearrange("(k p) m -> p k m", p=128),
        )
        win16 = const.tile([128, R], F16)
        nc.sync.dma_start(out=win16[:, :], in_=wi_d)
        b32 = const.tile([128, RC], F32)
        nc.sync.dma_start(out=b32[:, :], in_=b_d.rearrange("(m p) -> p m", p=128))

        # ---- big persistent buffers ----
        # zbuf m-major planes: col = m*(T*B) + t*B + b
        zbuf = big.tile([128, RC * T * B], F16)
        xT16 = big.tile([128, T * B], F16)
        h16 = big.tile([128, U * SUP], F16)
        nc.vector.memset(h16[:, (U - 1) * SUP:U * SUP], 0.0)

        zv = zbuf[:, :].rearrange("p (m t b) -> p m t b", t=T, b=B)
        hv = h16[:, :].rearrange("p (t m b) -> p t m b", m=RC, b=B)

        # ---- prepass: DMA cast+reorder to (t,b,r) scratch, then xbar ----
        nscr = dram.tile([T * B, R], F16)
        xscr = dram.tile([T * B, NIN], F16)
        nv = nscr[:, :].rearrange("(t b) r -> t b r", b=B)
        xv_s = xscr[:, :].rearrange("(t b) r -> t b r", b=B)
        ps_z = ctx.enter_context(tc.tile_pool(name="psz", bufs=2, space="PSUM"))
        PIECES = [(0, min(128, T))]
        if T > 128:
            PIECES.append((128, min(448, T)))
        if T > 448:
            PIECES.append((448, T))
        for (t0, t1) in PIECES:
            for b in range(B):
                nc.gpsimd.dma_start(out=nv[t0:t1, b, :], in_=n_d[b, t0:t1, :])
                nc.gpsimd.dma_start(out=xv_s[t0:t1, b, :], in_=x_d[b, t0:t1, :])
        for (t0, t1) in PIECES:
            for m in range(RC):
                nc.sync.dma_start(
                    out=zv[:, m, t0:t1, :].rearrange("p t b -> p (t b)"),
                    in_=nscr[t0 * B:t1 * B, m * 128:(m + 1) * 128],
                    transpose=True,
                )
            nc.sync.dma_start(out=xT16[:, t0 * B:t1 * B],
                              in_=xscr[t0 * B:t1 * B, :], transpose=True)

        def emit_prepass_zmm(p0, p1):
            # zbuf += x @ w_in + b_rec (0.2 applied in RELU_ADD_SC)
            for z0 in range(p0, p1, ZB):
                nt = min(ZB, p1 - z0)
                for m in range(RC):
                    zps = ps_z.tile([128, ZB * B], F32, tag="zps")
                    nc.tensor.matmul(
                        zps[:, :nt * B],
                        lhsT=win16[:, m * 128:(m + 1) * 128],
                        rhs=xT16[:, z0 * B:(z0 + nt) * B],
                        start=True, stop=True,
                    )
                    zsl = zv[:, m, z0:z0 + nt, :]
                    nc.vector.scalar_tensor_tensor(
                        out=zsl,
                        in0=zps[:, :nt * B].rearrange("p (t b) -> p t b", b=B),
                        scalar=b32[:, m:m + 1], in1=zsl,
                        op0=AOP.add, op1=AOP.add,
                    )

        # ---- recurrence + interleaved output drain ----
        # psum-resident recurrence: p_{t+1} = 0.8*p_t + r_t @ W
        #   r_t = relu((p_t + z_t) * 0.2)     (fp16, feeds next burst)
        #   h_t = 0.8*h_{t-1} + r_t           (fp16, output only)
        # Burst order per step: A=[k01 all m] C1=[m01 k23] I01 C2=[m23 k23] I23
        # so RA_a (banks m01) can run while PE does C2/I23.
        with tc.tile_pool(name="rp", bufs=2) as rp, \
             tc.tile_pool(name="sp", bufs=2) as sp, \
             tc.tile_pool(name="ostg", bufs=3) as ostg, \
             tc.tile_pool(name="psA0", bufs=1, space="PSUM") as ps_a0, \
             tc.tile_pool(name="psA1", bufs=1, space="PSUM") as ps_a1, \
             tc.tile_pool(name="psC0", bufs=1, space="PSUM") as ps_c0, \
             tc.tile_pool(name="psC1", bufs=1, space="PSUM") as ps_c1, \
             tc.tile_pool(name="psot", bufs=2, space="PSUM") as ps_ot:
            # one bank per half; two m-chunks at 128-col offsets; ping-pong
            # across step parity so a new burst never WARs pending readers.
            psAs = [ps_a0.tile([128, 512], F32, name="psa0", tag="psa0"),
                    ps_a1.tile([128, 512], F32, name="psa1", tag="psa1")]
            psCs = [ps_c0.tile([128, 512], F32, name="psc0", tag="psc0"),
                    ps_c1.tile([128, 512], F32, name="psc1", tag="psc1")]
            pvAs = [p[:, :].rearrange("p (m c) -> p m c", c=128) for p in psAs]
            pvCs = [p[:, :].rearrange("p (m c) -> p m c", c=128) for p in psCs]

            zero16 = const.tile([128, B], F16)
            nc.vector.memset(zero16[:, :], 0.0)

            def ps_of(m, par):
                ps = psAs[par] if m < 2 else psCs[par]
                return ps, (m % 2) * 128

            pending = []

            def emit_out_unit(u):
                blk_t0, nt, b, m = u
                rt0 = blk_t0 % U
                tp = ps_ot.tile([128, OBLK], F16, tag="otp")
                tr = nc.tensor.transpose(tp[:nt, :128],
                                         hv[:, rt0:rt0 + nt, m, b],
                                         ident16[:, :])
                st = ostg.tile([128, 128], F32, tag="ost")
                nc.scalar.copy(out=st[:nt, :], in_=tp[:nt, :128])
                nc.sync.dma_start(
                    out=o_d[b, blk_t0:blk_t0 + nt, m * 128:(m + 1) * 128],
                    in_=st[:nt, :],
                )
                return tr

            # prime p_0 = 0 (parity 0 banks; m%2==0 start clears the bank,
            # m%2==1 then overwrites via cleared has_written bits)
            for m in range(RC):
                ps, off = ps_of(m, 0)
                nc.tensor.matmul(ps[:, off:off + B], lhsT=w16[:, 0:128],
                                 rhs=zero16[:, :], start=(m % 2 == 0),
                                 stop=True, skip_group_check=True)

            emit_prepass_zmm(*PIECES[0])
            prev_ra = prev_rb = prev_sa = prev_sb = None
            for t in range(T):
                for pi in range(1, len(PIECES)):
                    if t == PIECES[pi][0] - 64:
                        emit_prepass_zmm(*PIECES[pi])
                rd = ((t - 1) % U) * SUP
                wr = (t % U) * SUP
                r16a = rp.tile([128, 16], F16, tag="r16a")
                r16b = rp.tile([128, 16], F16, tag="r16b")
                s16a = sp.tile([128, 16], F16, tag="s16a")
                s16b = sp.tile([128, 16], F16, tag="s16b")
                par = t % 2
                if t > 0:
                    def kmm(m, k, start=False, stop=False):
                        ps, off = ps_of(m, par)
                        src = prev_ra if k < 2 else prev_rb
                        return nc.tensor.matmul(
                            ps[:, off:off + B],
                            lhsT=w16[:, k * R + m * 128:k * R + (m + 1) * 128],
                            rhs=src[:, (k % 2) * B:(k % 2 + 1) * B],
                            start=start, stop=stop, skip_group_check=True,
                        )

                    def imm(m):
                        ps, off = ps_of(m, par)
                        src = prev_sa if m < 2 else prev_sb
                        return nc.tensor.matmul(
                            ps[:, off:off + B], lhsT=ident16[:, :],
                            rhs=src[:, (m % 2) * B:(m % 2 + 1) * B],
                            start=False, stop=True, skip_group_check=True,
                        )

                    for k in (0, 1):              # A: k01, all m
                        for m in range(RC):
                            kmm(m, k, start=(k == 0 and m % 2 == 0))
                    for m in (0, 1):              # C1: m01 k23
                        kmm(m, 2)
                        kmm(m, 3)
                    imm(0)                        # I01
                    i01_last = imm(1)
                    first_c2 = kmm(2, 2)          # C2: m23 k23
                    tile.add_dep_helper(
                        first_c2.ins, i01_last.ins, sync=False,
                        reason="keep I01 before C2 so RA_a unblocks early")
                    kmm(2, 3)
                    kmm(3, 2)
                    kmm(3, 3)
                    imm(2)                        # I23
                    last_mm = imm(3)

                # RA halves (DVE) + 0.8*p copies (ACT)
                nc.vector._custom_dve(
                    RELU_ADD_SC,
                    out=r16a[:, :].rearrange("p (m c) -> p m c", c=B),
                    in0=pvAs[par][:, 0:2, 0:B], in1=zv[:, 0:2, t, :],
                    s0=ALPHA)
                nc.scalar.mul(
                    out=s16a[:, :].rearrange("p (m c) -> p m c", c=B),
                    in_=pvAs[par][:, 0:2, 0:B], mul=LEAK)
                nc.vector._custom_dve(
                    RELU_ADD_SC,
                    out=r16b[:, :].rearrange("p (m c) -> p m c", c=B),
                    in0=pvCs[par][:, 0:2, 0:B], in1=zv[:, 2:4, t, :],
                    s0=ALPHA)
                nc.scalar.mul(
                    out=s16b[:, :].rearrange("p (m c) -> p m c", c=B),
                    in_=pvCs[par][:, 0:2, 0:B], mul=LEAK)
                # h output (off critical path)
                nc.vector.scalar_tensor_tensor(
                    out=h16[:, wr:wr + 16], in0=h16[:, rd:rd + 16],
                    scalar=LEAK, in1=r16a[:, :],
                    op0=AOP.mult, op1=AOP.add,
                )
                nc.vector.scalar_tensor_tensor(
                    out=h16[:, wr + 16:wr + SUP], in0=h16[:, rd + 16:rd + SUP],
                    scalar=LEAK, in1=r16b[:, :],
                    op0=AOP.mult, op1=AOP.add,
                )
                prev_ra, prev_rb = r16a, r16b
                prev_sa, prev_sb = s16a, s16b
                if (t + 1) % OBLK == 0 or t == T - 1:
                    blk_t0 = (t // OBLK) * OBLK
                    for b in range(B):
                        for m in range(RC):
                            pending.append((blk_t0, t + 1 - blk_t0, b, m))
                if pending and t >= OBLK:
                    emit_out_unit(pending.pop(0))
            while pending:
                emit_out_unit(pending.pop(0))

    if use_bacc:
        nc.compile()
    return nc


def host_prep(x, w_in, w_rec, b_rec, ei_mask, autapse_mask, noise):
    """Host-side weight prep + batch shard. Returns list of per-core in_maps."""
    ei = np.diagonal(np.asarray(ei_mask)).astype(np.float32)
    w_eff = ei[:, None] * (np.asarray(w_rec) * np.asarray(autapse_mask))
    w16 = w_eff.astype(np.float16)
    win16 = np.asarray(w_in).astype(np.float16)
    b32 = np.asarray(b_rec).astype(np.float32)
    x = np.asarray(x, dtype=np.float32)
    noise = np.asarray(noise, dtype=np.float32)
    bs = x.shape[0] // N_CORES
    in_maps = []
    for c in range(N_CORES):
        in_maps.append({
            "x_c": np.ascontiguousarray(x[c * bs:(c + 1) * bs]),
            "noise_c": np.ascontiguousarray(noise[c * bs:(c + 1) * bs]),
            "w16": w16,
            "win16": win16,
            "b32": b32,
        })
    return in_maps, w_eff.astype(np.float32)


def reference_np(x, w_in, b_rec, w_eff, noise, T=None):
    """Numpy reference for dev checks (f32)."""
    x = np.asarray(x, np.float32)
    if T is None:
        T = x.shape[1]
    z = np.einsum("bti,ir->btr", x[:, :T], np.asarray(w_in)) \
        + np.asarray(noise)[:, :T] + np.asarray(b_rec)
    h = np.zeros((x.shape[0], w_eff.shape[0]), np.float32)
    outs = []
    for t in range(T):
        pre = z[:, t] + h @ w_eff
        h = LEAK * h + ALPHA * np.maximum(pre, 0.0)
        outs.append(h.copy())
    return np.stack(outs, axis=1)


# ---------------------------------------------------------------------------
# harness entry point
# ---------------------------------------------------------------------------
_NC_CACHE = {}


def kernel(x, w_in, w_rec, b_rec, ei_mask, autapse_mask, noise):
    from concourse.bass_utils import run_bass_kernel_spmd

    x = np.asarray(x)
    T = x.shape[1]
    in_maps, _ = host_prep(x, w_in, w_rec, b_rec, ei_mask, autapse_mask, noise)
    if T not in _NC_CACHE:
        _NC_CACHE[T] = build_nc(T=T)
    nc = _NC_CACHE[T]
    res = run_bass_kernel_spmd(nc, in_maps, core_ids=list(range(N_CORES)))
    out = np.concatenate([r["out_c"] for r in res.results], axis=0)
    return out.astype(np.float32)



# revision 5
# speedup vs baseline: 1.6280x; 1.6280x over previous
"""BioRNN Trainium2 kernel (dev module).

Hybrid sharding: batch x2 (32 per core) and time x4 (250-step output
windows). The leak (0.8/step) makes the state forget: starting a window
100 steps early from h=0 reproduces the true state to ~1e-5 rel, so the
4 time-shards run independently with a 100-step burn-in (core qt=0 pads
z with zeros, exact). Per core: T=350 steps, B=32 batch.

Per-core math (fp16 weights/state):
    z_t  = x_t @ w_in + b_rec + noise_t                  (prepass, fp16)
    p_t  = h_{t-1} @ w_eff  (psum-resident, ping-pong parity banks)
    r_t  = relu((p_t + z_t) * 0.2)   (custom DVE op, fp16)
    p_{t+1} = 0.8*p_t + r_t @ w_eff  (0.8*p via ACT mul -> identity matmul)
    h_t  = 0.8*h_{t-1} + r_t         (DVE STT, h16 ring, output only)

Layouts (partition dim = r-chunk of 128; 4 chunks m=0..3):
  zbuf  sbuf fp16 (128, RC*T*B)  col = m*(T*B) + t*B + b
  h16   sbuf fp16 (128, U*SUP)   ring of U=128 steps, col = t*SUP+m*B+b
  w16   sbuf fp16 (128, 4*512)   [p, k*512 + m*128 + c] = W[k*128+p, m*128+c]
  xT16  sbuf fp16 (128, T*B)     col = t*B + b  (n_in on partitions)

Recurrence step: 16 matmuls (lhsT = w16 chunk (128,128), rhs = r16 slice
(128,B)) + 4 identity matmuls injecting 0.8*p, accumulating into parity
psum banks (2 halves x 2 parities). RA halves (custom DVE) + ACT muls.

Output: per 64-step block and batch b: 4 PE transposes of h16 into one
psum fp16 tile (t-part, 512 r-cols), then a single DMA (fp16->f32 cast)
straight to out[b, t0:t1, :].
"""

import numpy as np
from contextlib import ExitStack

import concourse.bass as bass
import concourse.mybir as mybir
import concourse.tile as tile
from concourse import bacc
from concourse import dve_ops
from concourse.dve_spec import Spec, Src0, Src1, C0, relu as _dve_relu_expr, lower
from concourse.dve_uop import DveOpSpec
from concourse.masks import make_identity


def _register_relu_add_sc():
    """Register fused out = relu((in0 + in1) * s0) custom DVE op (idempotent)."""
    name = "RELU_ADD_SC_BIO"
    for o in dve_ops.OPS:
        if o.name == name:
            return o
    opcode = max(dve_ops._SUB_OPCODE_FOR_NAME.values()) + 1
    assert opcode < 0x20
    dve_ops._SUB_OPCODE_FOR_NAME[name] = opcode

    def _ref(in0, in1, c0, c1, c2):
        a = in0.astype(np.float32).reshape(in0.shape[0], -1)
        b = in1.astype(np.float32).reshape(in1.shape[0], -1)
        s = np.maximum(np.nan_to_num((a + b) * c0, nan=0.0, posinf=np.inf,
                                     neginf=-np.inf), 0)
        return s.reshape(in0.shape)

    spec = Spec(body=_dve_relu_expr((Src0 + Src1) * C0), reference=_ref)
    shas = {}
    for ver in ("v3", "v4"):
        s = DveOpSpec(name=name, opcode=opcode, uops=lower(spec, ver=ver),
                      rd1_en=True)
        shas[ver] = s.sha(ver)
    op = dve_ops.DveOp(name, spec, subdim=False, uops_sha=shas)
    dve_ops.OPS.append(op)
    dve_ops.CUSTOM_DVE_SPECS[name] = spec
    return op


RELU_ADD_SC = _register_relu_add_sc()

F32 = mybir.dt.float32
F16 = mybir.dt.float16
AOP = mybir.AluOpType

R = 512          # n_rec
NIN = 128        # n_in
RC = 4           # r chunks (m and k)
N_CORES = 8
TSPLIT = 4       # time shards
BSPLIT = 2       # batch shards
B = 64 // BSPLIT          # batch per core
SUP = RC * B              # cols per step supertile
T_FULL = 1000
T_OUT = T_FULL // TSPLIT  # output steps per core
BURN = 100                # burn-in steps (truncation err ~1e-5)
T_LOC = T_OUT + BURN      # local steps per core
OUT0 = BURN               # first local step that produces output
ALPHA = 0.2
LEAK = 1.0 - ALPHA


def build_nc(T=T_LOC, U=128, use_bacc=True):
    """Build the per-core Bass program. U = h-ring steps."""
    OBLK = 64   # output block (steps); blocks align to 64 so ring slices
    assert U % (2 * OBLK) == 0   # never wrap
    nc = bacc.Bacc() if use_bacc else bass.Bass()

    x_d = nc.dram_tensor("x_c", [B, T, NIN], F32, kind="ExternalInput").ap()
    n_d = nc.dram_tensor("noise_c", [B, T, R], F32, kind="ExternalInput").ap()
    w_d = nc.dram_tensor("w16", [R, R], F16, kind="ExternalInput").ap()
    wi_d = nc.dram_tensor("win16", [NIN, R], F16, kind="ExternalInput").ap()
    b_d = nc.dram_tensor("b32", [R], F32, kind="ExternalInput").ap()
    o_d = nc.dram_tensor("out_c", [B, T_OUT, R], F32, kind="ExternalOutput").ap()

    ZB = 16  # zmm steps per prepass matmul (16*B = 512 moving cols)

    with tile.TileContext(nc) as tc, ExitStack() as ctx:
        const = ctx.enter_context(tc.tile_pool(name="const", bufs=1))
        big = ctx.enter_context(tc.tile_pool(name="big", bufs=1))
        dram = ctx.enter_context(tc.tile_pool(name="dram", bufs=1, space="DRAM"))

        # ---- constants ----
        ident16 = const.tile([128, 128], F16)
        make_identity(nc, ident16[:, :])

        w16 = const.tile([128, RC * R], F16)
        nc.sync.dma_start(
            out=w16[:, :].rearrange("p (k m) -> p k m", m=R),
            in_=w_d.rearrange("(k p) m -> p k m", p=128),
        )
        win16 = const.tile([128, R], F16)
        nc.sync.dma_start(out=win16[:, :], in_=wi_d)
        b32 = const.tile([128, RC], F32)
        nc.sync.dma_start(out=b32[:, :], in_=b_d.rearrange("(m p) -> p m", p=128))

        # ---- big persistent buffers ----
        zbuf = big.tile([128, RC * T * B], F16)
        xT16 = big.tile([128, T * B], F16)
        h16 = big.tile([128, U * SUP], F16)
        nc.vector.memset(h16[:, (U - 1) * SUP:U * SUP], 0.0)

        zv = zbuf[:, :].rearrange("p (m t b) -> p m t b", t=T, b=B)
        hv = h16[:, :].rearrange("p (t m b) -> p t m b", m=RC, b=B)

        # ---- prepass: DMA cast+reorder to (t,b,r) scratch, then xbar ----
        nscr = dram.tile([T * B, R], F16)
        xscr = dram.tile([T * B, NIN], F16)
        nv = nscr[:, :].rearrange("(t b) r -> t b r", b=B)
        xv_s = xscr[:, :].rearrange("(t b) r -> t b r", b=B)
        ps_z = ctx.enter_context(tc.tile_pool(name="psz", bufs=2, space="PSUM"))
        PIECES = [(0, 64), (64, 208), (208, T)]
        for (t0, t1) in PIECES:
            for b in range(B):
                nc.gpsimd.dma_start(out=nv[t0:t1, b, :], in_=n_d[b, t0:t1, :])
                nc.gpsimd.dma_start(out=xv_s[t0:t1, b, :], in_=x_d[b, t0:t1, :])
        for (t0, t1) in PIECES:
            for m in range(RC):
                nc.sync.dma_start(
                    out=zv[:, m, t0:t1, :].rearrange("p t b -> p (t b)"),
                    in_=nscr[t0 * B:t1 * B, m * 128:(m + 1) * 128],
                    transpose=True,
                )
            nc.sync.dma_start(out=xT16[:, t0 * B:t1 * B],
                              in_=xscr[t0 * B:t1 * B, :], transpose=True)

        def emit_prepass_zmm(p0, p1):
            # zbuf += x @ w_in + b_rec (0.2 applied in RELU_ADD_SC)
            for z0 in range(p0, p1, ZB):
                nt = min(ZB, p1 - z0)
                for m in range(RC):
                    zps = ps_z.tile([128, ZB * B], F32, tag="zps")
                    nc.tensor.matmul(
                        zps[:, :nt * B],
                        lhsT=win16[:, m * 128:(m + 1) * 128],
                        rhs=xT16[:, z0 * B:(z0 + nt) * B],
                        start=True, stop=True,
                    )
                    zsl = zv[:, m, z0:z0 + nt, :]
                    nc.vector.scalar_tensor_tensor(
                        out=zsl,
                        in0=zps[:, :nt * B].rearrange("p (t b) -> p t b", b=B),
                        scalar=b32[:, m:m + 1], in1=zsl,
                        op0=AOP.add, op1=AOP.add,
                    )

        # ---- output blocks: aligned to OBLK-step ring boundaries ----
        # OUT0=100, OBLK=64: starts [100, 128, 192, 256, 320]
        if OUT0 % OBLK:
            starts = [OUT0] + list(range(OUT0 + OBLK - OUT0 % OBLK, T, OBLK))
        else:
            starts = list(range(OUT0, T, OBLK))
        blocks = list(zip(starts, starts[1:] + [T]))

        # ---- recurrence + interleaved output drain ----
        with tc.tile_pool(name="rp", bufs=2) as rp, \
             tc.tile_pool(name="sp", bufs=2) as sp, \
             tc.tile_pool(name="ostg", bufs=3) as ostg, \
             tc.tile_pool(name="psA0", bufs=1, space="PSUM") as ps_a0, \
             tc.tile_pool(name="psA1", bufs=1, space="PSUM") as ps_a1, \
             tc.tile_pool(name="psC0", bufs=1, space="PSUM") as ps_c0, \
             tc.tile_pool(name="psC1", bufs=1, space="PSUM") as ps_c1, \
             tc.tile_pool(name="psot", bufs=2, space="PSUM") as ps_ot:
            psAs = [ps_a0.tile([128, 512], F32, name="psa0", tag="psa0"),
                    ps_a1.tile([128, 512], F32, name="psa1", tag="psa1")]
            psCs = [ps_c0.tile([128, 512], F32, name="psc0", tag="psc0"),
                    ps_c1.tile([128, 512], F32, name="psc1", tag="psc1")]
            pvAs = [p[:, :2 * B].rearrange("p (m c) -> p m c", c=B) for p in psAs]
            pvCs = [p[:, :2 * B].rearrange("p (m c) -> p m c", c=B) for p in psCs]

            zero16 = const.tile([128, B], F16)
            nc.vector.memset(zero16[:, :], 0.0)

            def ps_of(m, par):
                ps = psAs[par] if m < 2 else psCs[par]
                return ps, (m % 2) * B

            pending = []

            def emit_out_unit(u):
                blk_t0, nt, b = u
                rt0 = blk_t0 % U
                tp = ps_ot.tile([128, 512], F16, tag="otp")
                for m in range(RC):
                    nc.tensor.transpose(tp[:nt, m * 128:(m + 1) * 128],
                                        hv[:, rt0:rt0 + nt, m, b],
                                        ident16[:, :])
                st = ostg.tile([128, 512], F32, tag="ost")
                nc.scalar.copy(out=st[:nt, :], in_=tp[:nt, :])
                g0 = blk_t0 - OUT0
                nc.sync.dma_start(out=o_d[b, g0:g0 + nt, :], in_=st[:nt, :])

            # prime p_0 = 0 (parity 0 banks)
            for m in range(RC):
                ps, off = ps_of(m, 0)
                nc.tensor.matmul(ps[:, off:off + B], lhsT=w16[:, 0:128],
                                 rhs=zero16[:, :], start=(m % 2 == 0),
                                 stop=True, skip_group_check=True)

            emit_prepass_zmm(*PIECES[0])
            prev_ra = prev_rb = prev_sa = prev_sb = None
            for t in range(T):
                for pi in range(1, len(PIECES)):
                    if t == PIECES[pi][0] - 48:
                        emit_prepass_zmm(*PIECES[pi])
                rd = ((t - 1) % U) * SUP
                wr = (t % U) * SUP
                r16a = rp.tile([128, 2 * B], F16, tag="r16a")
                r16b = rp.tile([128, 2 * B], F16, tag="r16b")
                s16a = sp.tile([128, 2 * B], F16, tag="s16a")
                s16b = sp.tile([128, 2 * B], F16, tag="s16b")
                par = t % 2
                if t > 0:
                    def kmm(m, k, start=False, stop=False):
                        ps, off = ps_of(m, par)
                        src = prev_ra if k < 2 else prev_rb
                        return nc.tensor.matmul(
                            ps[:, off:off + B],
                            lhsT=w16[:, k * R + m * 128:k * R + (m + 1) * 128],
                            rhs=src[:, (k % 2) * B:(k % 2 + 1) * B],
                            start=start, stop=stop, skip_group_check=True,
                        )

                    def imm(m):
                        ps, off = ps_of(m, par)
                        src = prev_sa if m < 2 else prev_sb
                        return nc.tensor.matmul(
                            ps[:, off:off + B], lhsT=ident16[:, :],
                            rhs=src[:, (m % 2) * B:(m % 2 + 1) * B],
                            start=False, stop=True, skip_group_check=True,
                        )

                    for k in (0, 1):              # A: k01, all m
                        for m in range(RC):
                            kmm(m, k, start=(k == 0 and m % 2 == 0))
                    for m in (0, 1):              # C1: m01 k23
                        kmm(m, 2)
                        kmm(m, 3)
                    imm(0)                        # I01
                    i01_last = imm(1)
                    first_c2 = kmm(2, 2)          # C2: m23 k23
                    tile.add_dep_helper(
                        first_c2.ins, i01_last.ins, sync=False,
                        reason="keep I01 before C2 so RA_a unblocks early")
                    kmm(2, 3)
                    kmm(3, 2)
                    kmm(3, 3)
                    imm(2)                        # I23
                    imm(3)

                # RA halves (DVE) + 0.8*p copies (ACT)
                nc.vector._custom_dve(
                    RELU_ADD_SC,
                    out=r16a[:, :].rearrange("p (m c) -> p m c", c=B),
                    in0=pvAs[par][:, 0:2, 0:B], in1=zv[:, 0:2, t, :],
                    s0=ALPHA)
                nc.scalar.mul(
                    out=s16a[:, :].rearrange("p (m c) -> p m c", c=B),
                    in_=pvAs[par][:, 0:2, 0:B], mul=LEAK)
                nc.vector._custom_dve(
                    RELU_ADD_SC,
                    out=r16b[:, :].rearrange("p (m c) -> p m c", c=B),
                    in0=pvCs[par][:, 0:2, 0:B], in1=zv[:, 2:4, t, :],
                    s0=ALPHA)
                nc.scalar.mul(
                    out=s16b[:, :].rearrange("p (m c) -> p m c", c=B),
                    in_=pvCs[par][:, 0:2, 0:B], mul=LEAK)
                # h output (off critical path)
                nc.vector.scalar_tensor_tensor(
                    out=h16[:, wr:wr + 2 * B], in0=h16[:, rd:rd + 2 * B],
                    scalar=LEAK, in1=r16a[:, :],
                    op0=AOP.mult, op1=AOP.add,
                )
                nc.vector.scalar_tensor_tensor(
                    out=h16[:, wr + 2 * B:wr + SUP],
                    in0=h16[:, rd + 2 * B:rd + SUP],
                    scalar=LEAK, in1=r16b[:, :],
                    op0=AOP.mult, op1=AOP.add,
                )
                prev_ra, prev_rb = r16a, r16b
                prev_sa, prev_sb = s16a, s16b
                for (s, e) in blocks:
                    if t == e - 1:
                        for b in range(B):
                            pending.append((s, e - s, b))
                if pending and t > blocks[0][1]:
                    emit_out_unit(pending.pop(0))
            while pending:
                emit_out_unit(pending.pop(0))

    if use_bacc:
        nc.compile()
    return nc


def host_prep(x, w_in, w_rec, b_rec, ei_mask, autapse_mask, noise):
    """Host-side weight prep + hybrid batch/time shard with burn-in pad."""
    ei = np.diagonal(np.asarray(ei_mask)).astype(np.float32)
    w_eff = ei[:, None] * (np.asarray(w_rec) * np.asarray(autapse_mask))
    w16 = w_eff.astype(np.float16)
    win16 = np.asarray(w_in).astype(np.float16)
    b32 = np.asarray(b_rec).astype(np.float32)
    x = np.asarray(x, dtype=np.float32)
    noise = np.asarray(noise, dtype=np.float32)
    in_maps = []
    for c in range(N_CORES):
        beta, qt = c // TSPLIT, c % TSPLIT
        bs = slice(beta * B, (beta + 1) * B)
        t0 = qt * T_OUT - BURN
        xp = np.zeros((B, T_LOC, NIN), np.float32)
        npad = np.zeros((B, T_LOC, R), np.float32)
        s = max(t0, 0)
        off = s - t0
        xp[:, off:] = x[bs, s:t0 + T_LOC]
        npad[:, off:] = noise[bs, s:t0 + T_LOC]
        in_maps.append({
            "x_c": np.ascontiguousarray(xp),
            "noise_c": np.ascontiguousarray(npad),
            "w16": w16,
            "win16": win16,
            "b32": b32,
        })
    return in_maps, w_eff.astype(np.float32)


def reference_np(x, w_in, b_rec, w_eff, noise, T=None):
    """Numpy reference for dev checks (f32)."""
    x = np.asarray(x, np.float32)
    if T is None:
        T = x.shape[1]
    z = np.einsum("bti,ir->btr", x[:, :T], np.asarray(w_in)) \
        + np.asarray(noise)[:, :T] + np.asarray(b_rec)
    h = np.zeros((x.shape[0], w_eff.shape[0]), np.float32)
    outs = []
    for t in range(T):
        pre = z[:, t] + h @ w_eff
        h = LEAK * h + ALPHA * np.maximum(pre, 0.0)
        outs.append(h.copy())
    return np.stack(outs, axis=1)


# ---------------------------------------------------------------------------
# harness entry point
# ---------------------------------------------------------------------------
_NC_CACHE = {}


def kernel(x, w_in, w_rec, b_rec, ei_mask, autapse_mask, noise):
    from concourse.bass_utils import run_bass_kernel_spmd

    x = np.asarray(x)
    T = x.shape[1]
    in_maps, _ = host_prep(x, w_in, w_rec, b_rec, ei_mask, autapse_mask, noise)
    if T not in _NC_CACHE:
        _NC_CACHE[T] = build_nc()
    nc = _NC_CACHE[T]
    res = run_bass_kernel_spmd(nc, in_maps, core_ids=list(range(N_CORES)))
    out = np.empty((x.shape[0], T, R), np.float32)
    for c in range(N_CORES):
        beta, qt = c // TSPLIT, c % TSPLIT
        out[beta * B:(beta + 1) * B,
            qt * T_OUT:(qt + 1) * T_OUT] = res.results[c]["out_c"]
    return out


# revision 6
# speedup vs baseline: 1.6590x; 1.0190x over previous
"""BioRNN Trainium2 kernel (dev module).

Hybrid sharding: batch x2 (32 per core) and time x4 (250-step output
windows). The leak (0.8/step) makes the state forget: starting a window
100 steps early from h=0 reproduces the true state to ~1e-5 rel, so the
4 time-shards run independently with a 100-step burn-in (core qt=0 pads
z with zeros, exact). Per core: T=350 steps, B=32 batch.

accum-q recurrence (fp16 weights/state, no per-step leak matmuls):
within a Q=32 step block (j = t % Q), psum holds q = 0.8^-j * p where
p_t = h_{t-1} @ w_eff. Then q_{t+1} = q_t + r'_t @ w_eff with
    r'_t = 0.8^-(j+1) * r_t = relu((q + 0.8^-j * z_t) * 0.25)   (DVE)
    h_t  = 0.8*h_{t-1} + 0.8^(j+1) * r'_t                       (DVE)
Every Q steps the bank is re-injected at true scale via ACT mul
(0.8^Q * q -> fp16) + identity matmuls with start=True. 16 W-matmuls
per step, zero leak matmuls in steady state.

Layouts (partition dim = r-chunk of 128; 4 chunks m=0..3):
  zbuf  sbuf fp16 (128, RC*T*B)  col = m*(T*B) + t*B + b
  h16   sbuf fp16 (128, U*SUP)   ring of U=128 steps, col = t*SUP+m*B+b
  w16   sbuf fp16 (128, 4*512)   [p, k*512 + m*128 + c] = W[k*128+p, m*128+c]
  xT16  sbuf fp16 (128, T*B)     col = t*B + b  (n_in on partitions)
  psum: bank A = q cols m0|m1 (2*B), bank C = q cols m2|m3.

Step order: bank-A mms first (k01 gated by r'a, k23 by r'b), so RA_a of
the next step unblocks after 8 matmuls; bank-C mms run in the shadow.

Output: per <=64-step block and batch b: 4 PE transposes of h16 into one
psum fp16 tile (t-part, 512 r-cols), ACT copy to f32 staging, one DMA.
"""

import numpy as np
from contextlib import ExitStack

import concourse.bass as bass
import concourse.mybir as mybir
import concourse.tile as tile
from concourse import bacc
from concourse import dve_ops
from concourse.dve_spec import (
    Spec, Src0, Src1, C0, C1, relu as _dve_relu_expr, lower,
)
from concourse.dve_uop import DveOpSpec
from concourse.masks import make_identity


def _register_dve(name, body, ref):
    """Register a custom DVE op (idempotent)."""
    for o in dve_ops.OPS:
        if o.name == name:
            return o
    opcode = max(dve_ops._SUB_OPCODE_FOR_NAME.values()) + 1
    assert opcode < 0x20
    dve_ops._SUB_OPCODE_FOR_NAME[name] = opcode
    spec = Spec(body=body, reference=ref)
    shas = {}
    for ver in ("v3", "v4"):
        s = DveOpSpec(name=name, opcode=opcode, uops=lower(spec, ver=ver),
                      rd1_en=True)
        shas[ver] = s.sha(ver)
    op = dve_ops.DveOp(name, spec, subdim=False, uops_sha=shas)
    dve_ops.OPS.append(op)
    dve_ops.CUSTOM_DVE_SPECS[name] = spec
    return op


def _f32(a):
    return a.astype(np.float32).reshape(a.shape[0], -1)


def _ref_relu_qz(in0, in1, c0, c1, c2):
    s = np.maximum(np.nan_to_num((_f32(in0) + _f32(in1) * c1) * c0,
                                 nan=0.0, posinf=np.inf, neginf=-np.inf), 0)
    return s.reshape(in0.shape)


def _ref_leak2(in0, in1, c0, c1, c2):
    return (_f32(in0) * c0 + _f32(in1) * c1).reshape(in0.shape)


RELU_QZ = _register_dve("RELU_QZ_BIO",
                        _dve_relu_expr((Src0 + Src1 * C1) * C0), _ref_relu_qz)
LEAK2 = _register_dve("LEAK2_BIO", Src0 * C0 + Src1 * C1, _ref_leak2)

F32 = mybir.dt.float32
F16 = mybir.dt.float16
AOP = mybir.AluOpType

R = 512          # n_rec
NIN = 128        # n_in
RC = 4           # r chunks (m and k)
N_CORES = 8
TSPLIT = 4       # time shards
BSPLIT = 2       # batch shards
B = 64 // BSPLIT          # batch per core
SUP = RC * B              # cols per step supertile
T_FULL = 1000
T_OUT = T_FULL // TSPLIT  # output steps per core
BURN = 100                # burn-in steps (truncation err ~1e-5)
T_LOC = T_OUT + BURN      # local steps per core
OUT0 = BURN               # first local step that produces output
ALPHA = 0.2
LEAK = 1.0 - ALPHA
Q = 32                    # accum-q rescale block


def build_nc(T=T_LOC, U=128, use_bacc=True):
    """Build the per-core Bass program. U = h-ring steps."""
    OBLK = 64   # output block (steps); blocks align so ring slices
    assert U % (2 * OBLK) == 0   # never wrap
    nc = bacc.Bacc() if use_bacc else bass.Bass()

    x_d = nc.dram_tensor("x_c", [B, T, NIN], F32, kind="ExternalInput").ap()
    n_d = nc.dram_tensor("noise_c", [B, T, R], F32, kind="ExternalInput").ap()
    w_d = nc.dram_tensor("w16", [R, R], F16, kind="ExternalInput").ap()
    wi_d = nc.dram_tensor("win16", [NIN, R], F16, kind="ExternalInput").ap()
    b_d = nc.dram_tensor("b32", [R], F32, kind="ExternalInput").ap()
    o_d = nc.dram_tensor("out_c", [B, T_OUT, R], F32, kind="ExternalOutput").ap()

    ZB = 16  # zmm steps per prepass matmul (16*B = 512 moving cols)

    with tile.TileContext(nc) as tc, ExitStack() as ctx:
        const = ctx.enter_context(tc.tile_pool(name="const", bufs=1))
        big = ctx.enter_context(tc.tile_pool(name="big", bufs=1))
        dram = ctx.enter_context(tc.tile_pool(name="dram", bufs=1, space="DRAM"))

        # ---- constants ----
        ident16 = const.tile([128, 128], F16)
        make_identity(nc, ident16[:, :])

        w16 = const.tile([128, RC * R], F16)
        nc.sync.dma_start(
            out=w16[:, :].rearrange("p (k m) -> p k m", m=R),
            in_=w_d.rearrange("(k p) m -> p k m", p=128),
        )
        win16 = const.tile([128, R], F16)
        nc.sync.dma_start(out=win16[:, :], in_=wi_d)
        b32 = const.tile([128, RC], F32)
        nc.sync.dma_start(out=b32[:, :], in_=b_d.rearrange("(m p) -> p m", p=128))

        # ---- big persistent buffers ----
        zbuf = big.tile([128, RC * T * B], F16)
        xT16 = big.tile([128, T * B], F16)
        h16 = big.tile([128, U * SUP], F16)
        nc.vector.memset(h16[:, (U - 1) * SUP:U * SUP], 0.0)

        zv = zbuf[:, :].rearrange("p (m t b) -> p m t b", t=T, b=B)
        hv = h16[:, :].rearrange("p (t m b) -> p t m b", m=RC, b=B)

        # ---- prepass: DMA cast+reorder to (t,b,r) scratch, then xbar ----
        nscr = dram.tile([T * B, R], F16)
        xscr = dram.tile([T * B, NIN], F16)
        nv = nscr[:, :].rearrange("(t b) r -> t b r", b=B)
        xv_s = xscr[:, :].rearrange("(t b) r -> t b r", b=B)
        ps_z = ctx.enter_context(tc.tile_pool(name="psz", bufs=2, space="PSUM"))
        PIECES = [(0, 24), (24, 88), (88, 216), (216, T)]
        for (t0, t1) in PIECES:
            for b in range(B):
                nc.gpsimd.dma_start(out=nv[t0:t1, b, :], in_=n_d[b, t0:t1, :])
                nc.gpsimd.dma_start(out=xv_s[t0:t1, b, :], in_=x_d[b, t0:t1, :])
        for (t0, t1) in PIECES:
            for m in range(RC):
                nc.sync.dma_start(
                    out=zv[:, m, t0:t1, :].rearrange("p t b -> p (t b)"),
                    in_=nscr[t0 * B:t1 * B, m * 128:(m + 1) * 128],
                    transpose=True,
                )
            nc.sync.dma_start(out=xT16[:, t0 * B:t1 * B],
                              in_=xscr[t0 * B:t1 * B, :], transpose=True)

        def emit_prepass_zmm(p0, p1):
            # zbuf += x @ w_in + b_rec
            for z0 in range(p0, p1, ZB):
                nt = min(ZB, p1 - z0)
                for m in range(RC):
                    zps = ps_z.tile([128, ZB * B], F32, tag="zps")
                    nc.tensor.matmul(
                        zps[:, :nt * B],
                        lhsT=win16[:, m * 128:(m + 1) * 128],
                        rhs=xT16[:, z0 * B:(z0 + nt) * B],
                        start=True, stop=True,
                    )
                    zsl = zv[:, m, z0:z0 + nt, :]
                    nc.vector.scalar_tensor_tensor(
                        out=zsl,
                        in0=zps[:, :nt * B].rearrange("p (t b) -> p t b", b=B),
                        scalar=b32[:, m:m + 1], in1=zsl,
                        op0=AOP.add, op1=AOP.add,
                    )

        # ---- output blocks: ring slices must not cross 128-step lines ----
        # OUT0=100, OBLK=64: starts [100, 128, 192, 256, 320] + small tail
        if OUT0 % OBLK:
            starts = [OUT0] + list(range(OUT0 + OBLK - OUT0 % OBLK, T, OBLK))
        else:
            starts = list(range(OUT0, T, OBLK))
        if T - starts[-1] > 24:  # keep the post-loop drain small
            starts.append(T - 14)
        blocks = list(zip(starts, starts[1:] + [T]))

        # ---- recurrence + interleaved output drain ----
        with tc.tile_pool(name="rp", bufs=2) as rp, \
             tc.tile_pool(name="sp", bufs=2) as sp, \
             tc.tile_pool(name="ostg", bufs=3) as ostg, \
             tc.tile_pool(name="psA", bufs=1, space="PSUM") as ps_a, \
             tc.tile_pool(name="psC", bufs=1, space="PSUM") as ps_c, \
             tc.tile_pool(name="psot", bufs=2, space="PSUM") as ps_ot:
            psA = ps_a.tile([128, 512], F32, name="psa", tag="psa")
            psC = ps_c.tile([128, 512], F32, name="psc", tag="psc")
            pvA = psA[:, :2 * B].rearrange("p (m c) -> p m c", c=B)
            pvC = psC[:, :2 * B].rearrange("p (m c) -> p m c", c=B)

            zero16 = const.tile([128, B], F16)
            nc.vector.memset(zero16[:, :], 0.0)

            def ps_of(m):
                ps = psA if m < 2 else psC
                return ps, (m % 2) * B

            pending = []
            n_emitted = 0

            def emit_out_unit(u, tail=False):
                nonlocal n_emitted
                blk_t0, nt, b = u
                rt0 = blk_t0 % U
                tp = ps_ot.tile([128, 512], F16, tag="otp")
                for m in range(RC):
                    nc.tensor.transpose(tp[:nt, m * 128:(m + 1) * 128],
                                        hv[:, rt0:rt0 + nt, m, b],
                                        ident16[:, :])
                st = ostg.tile([128, 512], F32, tag="ost")
                if tail and n_emitted % 2:
                    nc.vector.tensor_scalar_add(st[:nt, :], tp[:nt, :], 0.0)
                else:
                    nc.scalar.copy(out=st[:nt, :], in_=tp[:nt, :])
                n_emitted += 1
                g0 = blk_t0 - OUT0
                nc.sync.dma_start(out=o_d[b, g0:g0 + nt, :], in_=st[:nt, :])

            # prime q = p_0 = 0
            for m in range(RC):
                ps, off = ps_of(m)
                nc.tensor.matmul(ps[:, off:off + B], lhsT=w16[:, 0:128],
                                 rhs=zero16[:, :], start=(m % 2 == 0),
                                 stop=(m % 2 == 1), skip_group_check=True)

            emit_prepass_zmm(*PIECES[0])
            prev_r = None
            for t in range(T):
                for pi in range(1, len(PIECES)):
                    if t == PIECES[pi][0] - 20:
                        emit_prepass_zmm(*PIECES[pi])
                rd = ((t - 1) % U) * SUP
                wr = (t % U) * SUP
                rbig = rp.tile([128, SUP], F16, tag="rbig")
                jp = t % Q          # frame of q after this iteration's mms
                jn = (t + 1) % Q    # frame after the next iteration's mms
                if t > 0:
                    if jp == 0:
                        # restart: re-inject q at true scale (q := 0.8^Q * q)
                        s16a = sp.tile([128, 2 * B], F16, tag="s16a")
                        s16b = sp.tile([128, 2 * B], F16, tag="s16b")
                        nc.scalar.mul(out=s16a[:, :], in_=psA[:, :2 * B],
                                      mul=float(LEAK ** Q))
                        nc.scalar.mul(out=s16b[:, :], in_=psC[:, :2 * B],
                                      mul=float(LEAK ** Q))
                        for m in range(RC):
                            ps, off = ps_of(m)
                            src = s16a if m < 2 else s16b
                            nc.tensor.matmul(
                                ps[:, off:off + B], lhsT=ident16[:, :],
                                rhs=src[:, (m % 2) * B:(m % 2 + 1) * B],
                                start=(m % 2 == 0), stop=False,
                                skip_group_check=True)

                    def kmm(m, k, stop=False):
                        ps, off = ps_of(m)
                        return nc.tensor.matmul(
                            ps[:, off:off + B],
                            lhsT=w16[:, k * R + m * 128:k * R + (m + 1) * 128],
                            rhs=prev_r[:, k * B:(k + 1) * B],
                            start=False, stop=stop, skip_group_check=True)

                    # bank A (m01) first: k01 (gated by r'a) then k23 (r'b)
                    kmm(0, 0); kmm(1, 0); kmm(0, 1); kmm(1, 1)
                    kmm(0, 2); kmm(1, 2); kmm(0, 3); kmm(1, 3, stop=True)
                    # bank C (m23) in the shadow
                    kmm(2, 0); kmm(3, 0); kmm(2, 1); kmm(3, 1)
                    kmm(2, 2); kmm(3, 2); kmm(2, 3); kmm(3, 3, stop=True)

                # r' = relu((q + 0.8^-jp * z) * 0.2*0.8^(jp-jn))   (DVE)
                s0 = float(ALPHA * LEAK ** (jp - jn))
                s1 = float(LEAK ** (-jp))
                nc.vector._custom_dve(
                    RELU_QZ,
                    out=rbig[:, :2 * B].rearrange("p (m c) -> p m c", c=B),
                    in0=pvA[:, 0:2, 0:B], in1=zv[:, 0:2, t, :],
                    s0=s0, s1=s1)
                nc.vector._custom_dve(
                    RELU_QZ,
                    out=rbig[:, 2 * B:].rearrange("p (m c) -> p m c", c=B),
                    in0=pvC[:, 0:2, 0:B], in1=zv[:, 2:4, t, :],
                    s0=s0, s1=s1)
                # h output: h_t = 0.8*h_{t-1} + 0.8^jn * r'  (off critical path)
                nc.vector._custom_dve(
                    LEAK2,
                    out=h16[:, wr:wr + SUP], in0=h16[:, rd:rd + SUP],
                    in1=rbig[:, :], s0=float(LEAK), s1=float(LEAK ** jn))
                prev_r = rbig
                for (s, e) in blocks:
                    if t == e - 1:
                        for b in range(B):
                            pending.append((s, e - s, b))
                if pending and t > blocks[0][1]:
                    emit_out_unit(pending.pop(0))
            while pending:
                emit_out_unit(pending.pop(0), tail=True)

    if use_bacc:
        nc.compile()
    return nc


def host_prep(x, w_in, w_rec, b_rec, ei_mask, autapse_mask, noise):
    """Host-side weight prep + hybrid batch/time shard with burn-in pad."""
    ei = np.diagonal(np.asarray(ei_mask)).astype(np.float32)
    w_eff = ei[:, None] * (np.asarray(w_rec) * np.asarray(autapse_mask))
    w16 = w_eff.astype(np.float16)
    win16 = np.asarray(w_in).astype(np.float16)
    b32 = np.asarray(b_rec).astype(np.float32)
    x = np.asarray(x, dtype=np.float32)
    noise = np.asarray(noise, dtype=np.float32)
    in_maps = []
    for c in range(N_CORES):
        beta, qt = c // TSPLIT, c % TSPLIT
        bs = slice(beta * B, (beta + 1) * B)
        t0 = qt * T_OUT - BURN
        xp = np.zeros((B, T_LOC, NIN), np.float32)
        npad = np.zeros((B, T_LOC, R), np.float32)
        s = max(t0, 0)
        off = s - t0
        xp[:, off:] = x[bs, s:t0 + T_LOC]
        npad[:, off:] = noise[bs, s:t0 + T_LOC]
        in_maps.append({
            "x_c": np.ascontiguousarray(xp),
            "noise_c": np.ascontiguousarray(npad),
            "w16": w16,
            "win16": win16,
            "b32": b32,
        })
    return in_maps, w_eff.astype(np.float32)


def reference_np(x, w_in, b_rec, w_eff, noise, T=None):
    """Numpy reference for dev checks (f32)."""
    x = np.asarray(x, np.float32)
    if T is None:
        T = x.shape[1]
    z = np.einsum("bti,ir->btr", x[:, :T], np.asarray(w_in)) \
        + np.asarray(noise)[:, :T] + np.asarray(b_rec)
    h = np.zeros((x.shape[0], w_eff.shape[0]), np.float32)
    outs = []
    for t in range(T):
        pre = z[:, t] + h @ w_eff
        h = LEAK * h + ALPHA * np.maximum(pre, 0.0)
        outs.append(h.copy())
    return np.stack(outs, axis=1)


# ---------------------------------------------------------------------------
# harness entry point
# ---------------------------------------------------------------------------
_NC_CACHE = {}


def kernel(x, w_in, w_rec, b_rec, ei_mask, autapse_mask, noise):
    from concourse.bass_utils import run_bass_kernel_spmd

    x = np.asarray(x)
    T = x.shape[1]
    in_maps, _ = host_prep(x, w_in, w_rec, b_rec, ei_mask, autapse_mask, noise)
    if T not in _NC_CACHE:
        _NC_CACHE[T] = build_nc()
    nc = _NC_CACHE[T]
    res = run_bass_kernel_spmd(nc, in_maps, core_ids=list(range(N_CORES)))
    out = np.empty((x.shape[0], T, R), np.float32)
    for c in range(N_CORES):
        beta, qt = c // TSPLIT, c % TSPLIT
        out[beta * B:(beta + 1) * B,
            qt * T_OUT:(qt + 1) * T_OUT] = res.results[c]["out_c"]
    return out


# revision 12
# speedup vs baseline: 3.0326x; 1.8280x over previous
"""BioRNN Trainium2 kernel (dev module).

Sharding: time x8 (125-step output windows, full batch 64 per core).
The leak (0.8/step) makes the state forget: starting a window 100 steps
early from h=0 reproduces the true state to ~1e-5 rel, so the 8 time
shards run independently with a 100-step burn-in (core 0 pads z with
zeros, exact). Per core: T=225 steps, B=64 batch.

accum-q recurrence (fp16 weights/state, no per-step leak matmuls):
within a Q=32 step block (j = t % Q), psum holds q = 0.8^-j * p where
p_t = h_{t-1} @ w_eff. Then q_{t+1} = q_t + r'_t @ w_eff with
    r'_t = 0.8^-(j+1) * r_t = relu((q + 0.8^-j * z_t) * 0.25)   (DVE)
    h_t  = 0.8*h_{t-1} + 0.8^(j+1) * r'_t                       (DVE)
Every Q steps the bank is re-injected at true scale via ACT mul
(0.8^Q * q -> fp16) + identity matmuls with start=True. 16 W-matmuls
per step, zero leak matmuls in steady state.

Inputs arrive host-pre-transposed fp16 (noiseT[p, m, t, b], xT[p, t, b])
so the prepass is just: DMA chunk into the z ring, then 4 matmuls
(x @ w_in) + 4 DVE adds per 16-step chunk, emitted spread-out, one
chunk ~24 steps ahead of use.

Layouts (partition dim = r-chunk of 128; 4 chunks m=0..3):
  zring sbuf fp16 (128, RC*ZR*B)  ring of ZR=128 steps of z
  h16   sbuf fp16 (128, U*SUP)    ring of U=128 steps, col = t*SUP+m*B+b
  w16   sbuf fp16 (128, 4*512)    [p, k*512+m*128+c] = W[k*128+p, m*128+c]
  xT16  sbuf fp16 (128, T*B)      col = t*B + b  (n_in on partitions)
  psum: bank A = q cols m0|m1 (2*B), bank C = q cols m2|m3.

Step order: bank-A mms first (k01 gated by r'a, k23 by r'b), so RA_a of
the next step unblocks after 8 matmuls; bank-C mms run in the shadow.

Output: per <=64-step block and batch PAIR (2b,2b+1): 8 PE transposes of
h16 into one psum fp16 tile (rows 0-63 / 64-127), one ACT copy to f32
staging, one DMA covering both batches.
"""

import numpy as np
from contextlib import ExitStack

import concourse.bass as bass
import concourse.mybir as mybir
import concourse.tile as tile
from concourse import bacc
from concourse import dve_ops
from concourse.dve_spec import (
    Spec, Src0, Src1, C0, C1, relu as _dve_relu_expr, lower,
)
from concourse.dve_uop import DveOpSpec
from concourse.masks import make_identity


def _register_dve(name, body, ref):
    """Register a custom DVE op (idempotent)."""
    for o in dve_ops.OPS:
        if o.name == name:
            return o
    opcode = max(dve_ops._SUB_OPCODE_FOR_NAME.values()) + 1
    assert opcode < 0x20
    dve_ops._SUB_OPCODE_FOR_NAME[name] = opcode
    spec = Spec(body=body, reference=ref)
    shas = {}
    for ver in ("v3", "v4"):
        s = DveOpSpec(name=name, opcode=opcode, uops=lower(spec, ver=ver),
                      rd1_en=True)
        shas[ver] = s.sha(ver)
    op = dve_ops.DveOp(name, spec, subdim=False, uops_sha=shas)
    dve_ops.OPS.append(op)
    dve_ops.CUSTOM_DVE_SPECS[name] = spec
    return op


def _f32(a):
    return a.astype(np.float32).reshape(a.shape[0], -1)


def _ref_relu_qz(in0, in1, c0, c1, c2):
    s = np.maximum(np.nan_to_num((_f32(in0) + _f32(in1) * c1) * c0,
                                 nan=0.0, posinf=np.inf, neginf=-np.inf), 0)
    return s.reshape(in0.shape)


def _ref_leak2(in0, in1, c0, c1, c2):
    return (_f32(in0) * c0 + _f32(in1) * c1).reshape(in0.shape)


RELU_QZ = _register_dve("RELU_QZ_BIO",
                        _dve_relu_expr((Src0 + Src1 * C1) * C0), _ref_relu_qz)
LEAK2 = _register_dve("LEAK2_BIO", Src0 * C0 + Src1 * C1, _ref_leak2)

F32 = mybir.dt.float32
F16 = mybir.dt.float16
AOP = mybir.AluOpType

R = 512          # n_rec
NIN = 128        # n_in
RC = 4           # r chunks (m and k)
N_CORES = 8
TSPLIT = 8       # time shards
B = 64           # batch per core (full batch)
SUP = RC * B     # cols per step supertile
T_FULL = 1000
T_OUT = T_FULL // TSPLIT  # output steps per core
BURN = 100                # burn-in steps (truncation err ~1e-5)
T_LOC = T_OUT + BURN      # local steps per core
OUT0 = BURN               # first local step that produces output
ALPHA = 0.2
LEAK = 1.0 - ALPHA
Q = 32                    # accum-q rescale block
ZR = 128                  # z ring steps
ZCH = 16                  # z chunk (DMA + zmm granularity)
ZLEAD = 32                # emit z chunk this many steps ahead


def build_nc(T=T_LOC, U=128, use_bacc=True):
    """Build the per-core Bass program. U = h-ring steps."""
    OBLK = 64
    assert U % (2 * OBLK) == 0
    nc = bacc.Bacc() if use_bacc else bass.Bass()

    # host-pre-transposed fp16 inputs
    xT_d = nc.dram_tensor("xT16", [NIN, T, B], F16, kind="ExternalInput").ap()
    nT_d = nc.dram_tensor("noiseT16", [128, RC, T, B], F16,
                          kind="ExternalInput").ap()
    w_d = nc.dram_tensor("w16", [R, R], F16, kind="ExternalInput").ap()
    wi_d = nc.dram_tensor("win16", [NIN, R], F16, kind="ExternalInput").ap()
    b_d = nc.dram_tensor("b32", [R], F32, kind="ExternalInput").ap()
    o_d = nc.dram_tensor("out_c", [B, T_OUT, R], F32, kind="ExternalOutput").ap()

    with tile.TileContext(nc) as tc, ExitStack() as ctx:
        const = ctx.enter_context(tc.tile_pool(name="const", bufs=1))
        big = ctx.enter_context(tc.tile_pool(name="big", bufs=1))

        # ---- constants ----
        ident16 = const.tile([128, 128], F16)
        make_identity(nc, ident16[:, :])

        w16 = const.tile([128, RC * R], F16)
        nc.sync.dma_start(
            out=w16[:, :].rearrange("p (k m) -> p k m", m=R),
            in_=w_d.rearrange("(k p) m -> p k m", p=128),
        )
        win16 = const.tile([128, R], F16)
        nc.sync.dma_start(out=win16[:, :], in_=wi_d)
        b32 = const.tile([128, RC], F32)
        nc.sync.dma_start(out=b32[:, :], in_=b_d.rearrange("(m p) -> p m", p=128))

        # ---- big persistent buffers ----
        zring = big.tile([128, RC * ZR * B], F16)
        xT16 = big.tile([128, T * B], F16)
        h16 = big.tile([128, U * SUP], F16)
        nc.vector.memset(h16[:, (U - 1) * SUP:U * SUP], 0.0)

        zv = zring[:, :].rearrange("p (m t b) -> p m t b", t=ZR, b=B)
        hv = h16[:, :].rearrange("p (t m b) -> p t m b", m=RC, b=B)

        # xT16 loaded in two pieces (first small for fast start)
        for (t0, t1) in ((0, ZCH), (ZCH, T)):
            nc.sync.dma_start(out=xT16[:, t0 * B:t1 * B],
                              in_=xT_d[:, t0:t1, :])

        ps_z = ctx.enter_context(tc.tile_pool(name="psz", bufs=2, space="PSUM"))
        ZB = 8  # zmm steps per matmul (8*B = 512 moving cols)

        def emit_z_chunk(t0):
            """DMA noise chunk into ring + z = x @ w_in + b + noise."""
            t1 = min(t0 + ZCH, T)
            rt = t0 % ZR
            nc.gpsimd.dma_start(out=zv[:, :, rt:rt + (t1 - t0), :],
                                in_=nT_d[:, :, t0:t1, :])
            for z0 in range(t0, t1, ZB):
                nt = min(ZB, t1 - z0)
                rz = z0 % ZR
                for m in range(RC):
                    zps = ps_z.tile([128, ZB * B], F32, tag="zps")
                    nc.tensor.matmul(
                        zps[:, :nt * B],
                        lhsT=win16[:, m * 128:(m + 1) * 128],
                        rhs=xT16[:, z0 * B:(z0 + nt) * B],
                        start=True, stop=True,
                    )
                    zsl = zv[:, m, rz:rz + nt, :]
                    nc.vector.scalar_tensor_tensor(
                        out=zsl,
                        in0=zps[:, :nt * B].rearrange("p (t b) -> p t b", b=B),
                        scalar=b32[:, m:m + 1], in1=zsl,
                        op0=AOP.add, op1=AOP.add,
                    )

        # ---- output blocks: ring slices must not cross 128-step lines ----
        if OUT0 % OBLK:
            starts = [OUT0] + list(range(OUT0 + OBLK - OUT0 % OBLK, T, OBLK))
        else:
            starts = list(range(OUT0, T, OBLK))
        if T - starts[-1] > 24:  # keep the post-loop drain small
            starts.append(T - 12)
        blocks = list(zip(starts, starts[1:] + [T]))

        # ---- recurrence + interleaved output drain ----
        with tc.tile_pool(name="rp", bufs=2) as rp, \
             tc.tile_pool(name="sp", bufs=2) as sp, \
             tc.tile_pool(name="ostg", bufs=4) as ostg, \
             tc.tile_pool(name="psA", bufs=1, space="PSUM") as ps_a, \
             tc.tile_pool(name="psC", bufs=1, space="PSUM") as ps_c, \
             tc.tile_pool(name="psot", bufs=3, space="PSUM") as ps_ot:
            psA = ps_a.tile([128, 512], F32, name="psa", tag="psa")
            psC = ps_c.tile([128, 512], F32, name="psc", tag="psc")
            pvA = psA[:, :2 * B].rearrange("p (m c) -> p m c", c=B)
            pvC = psC[:, :2 * B].rearrange("p (m c) -> p m c", c=B)

            zero16 = const.tile([128, B], F16)
            nc.vector.memset(zero16[:, :], 0.0)

            def ps_of(m):
                ps = psA if m < 2 else psC
                return ps, (m % 2) * B

            pending = []

            def emit_out_unit(u):
                """One unit = batch pair, column-stacked in one psum bank
                (PE output rows are hard-wired to psum partitions, so the
                pair must stack along columns, not partitions)."""
                blk_t0, nt, bp = u
                rt0 = blk_t0 % U
                tp = ps_ot.tile([128, 1024], F16, tag="otp")
                for i in (0, 1):
                    for m in range(RC):
                        nc.tensor.transpose(
                            tp[:nt, i * 512 + m * 128:i * 512 + (m + 1) * 128],
                            hv[:, rt0:rt0 + nt, m, 2 * bp + i],
                            ident16[:, :])
                st = ostg.tile([128, 1024], F32, tag="ost")
                nc.scalar.copy(out=st[:nt, :], in_=tp[:nt, :])
                g0 = blk_t0 - OUT0
                nc.sync.dma_start(
                    out=o_d[2 * bp:2 * bp + 2, g0:g0 + nt, :]
                        .rearrange("b t r -> t b r"),
                    in_=st[:nt, :].rearrange("t (i r) -> t i r", i=2))

            # prime q = p_0 = 0
            for m in range(RC):
                ps, off = ps_of(m)
                nc.tensor.matmul(ps[:, off:off + B], lhsT=w16[:, 0:128],
                                 rhs=zero16[:, :], start=(m % 2 == 0),
                                 stop=(m % 2 == 1), skip_group_check=True)

            emit_z_chunk(0)
            emit_z_chunk(ZCH)
            prev_r = None
            for t in range(T):
                if (t + ZLEAD) % ZCH == 0 and 2 * ZCH <= t + ZLEAD < T:
                    emit_z_chunk(t + ZLEAD)
                rd = ((t - 1) % U) * SUP
                wr = (t % U) * SUP
                rbig = rp.tile([128, SUP], F16, tag="rbig")
                jp = t % Q          # frame of q after this iteration's mms
                jn = (t + 1) % Q    # frame after the next iteration's mms
                if t > 0:
                    if jp == 0:
                        # restart: re-inject q at true scale (q := 0.8^Q * q)
                        s16a = sp.tile([128, 2 * B], F16, tag="s16a")
                        s16b = sp.tile([128, 2 * B], F16, tag="s16b")
                        nc.scalar.mul(out=s16a[:, :], in_=psA[:, :2 * B],
                                      mul=float(LEAK ** Q))
                        nc.scalar.mul(out=s16b[:, :], in_=psC[:, :2 * B],
                                      mul=float(LEAK ** Q))
                        for m in range(RC):
                            ps, off = ps_of(m)
                            src = s16a if m < 2 else s16b
                            nc.tensor.matmul(
                                ps[:, off:off + B], lhsT=ident16[:, :],
                                rhs=src[:, (m % 2) * B:(m % 2 + 1) * B],
                                start=(m % 2 == 0), stop=False,
                                skip_group_check=True)

                    def kmm(m, k, stop=False):
                        ps, off = ps_of(m)
                        return nc.tensor.matmul(
                            ps[:, off:off + B],
                            lhsT=w16[:, k * R + m * 128:k * R + (m + 1) * 128],
                            rhs=prev_r[:, k * B:(k + 1) * B],
                            start=False, stop=stop, skip_group_check=True)

                    # bank A (m01) first: k01 (gated by r'a) then k23 (r'b)
                    kmm(0, 0); kmm(1, 0); kmm(0, 1); kmm(1, 1)
                    kmm(0, 2); kmm(1, 2); kmm(0, 3); kmm(1, 3, stop=True)
                    # bank C (m23) in the shadow
                    kmm(2, 0); kmm(3, 0); kmm(2, 1); kmm(3, 1)
                    kmm(2, 2); kmm(3, 2); kmm(2, 3); kmm(3, 3, stop=True)

                # r' = relu((q + 0.8^-jp * z) * 0.2*0.8^(jp-jn))   (DVE)
                s0 = float(ALPHA * LEAK ** (jp - jn))
                s1 = float(LEAK ** (-jp))
                zt = t % ZR
                nc.vector._custom_dve(
                    RELU_QZ,
                    out=rbig[:, :2 * B].rearrange("p (m c) -> p m c", c=B),
                    in0=pvA[:, 0:2, 0:B], in1=zv[:, 0:2, zt, :],
                    s0=s0, s1=s1)
                nc.vector._custom_dve(
                    RELU_QZ,
                    out=rbig[:, 2 * B:].rearrange("p (m c) -> p m c", c=B),
                    in0=pvC[:, 0:2, 0:B], in1=zv[:, 2:4, zt, :],
                    s0=s0, s1=s1)
                # h output: h_t = 0.8*h_{t-1} + 0.8^jn * r'  (off critical path)
                nc.vector._custom_dve(
                    LEAK2,
                    out=h16[:, wr:wr + SUP], in0=h16[:, rd:rd + SUP],
                    in1=rbig[:, :], s0=float(LEAK), s1=float(LEAK ** jn))
                prev_r = rbig
                for (s, e) in blocks:
                    if t == e - 1:
                        for bp in range(B // 2):
                            pending.append((s, e - s, bp))
                if pending and t > blocks[0][1]:
                    emit_out_unit(pending.pop(0))
            while pending:
                emit_out_unit(pending.pop(0))

    if use_bacc:
        nc.compile()
    return nc


def host_prep(x, w_in, w_rec, b_rec, ei_mask, autapse_mask, noise):
    """Host-side weight prep + time shard + fp16 layout transpose."""
    ei = np.diagonal(np.asarray(ei_mask)).astype(np.float32)
    w_eff = ei[:, None] * (np.asarray(w_rec) * np.asarray(autapse_mask))
    w16 = w_eff.astype(np.float16)
    win16 = np.asarray(w_in).astype(np.float16)
    b32 = np.asarray(b_rec).astype(np.float32)
    x = np.asarray(x, dtype=np.float32)
    noise = np.asarray(noise, dtype=np.float32)
    in_maps = []
    for c in range(N_CORES):
        t0 = c * T_OUT - BURN
        xp = np.zeros((B, T_LOC, NIN), np.float16)
        npad = np.zeros((B, T_LOC, R), np.float16)
        s = max(t0, 0)
        off = s - t0
        xp[:, off:] = x[:, s:t0 + T_LOC]
        npad[:, off:] = noise[:, s:t0 + T_LOC]
        # xT16[p, t, b] = x[b, t, p]
        xT = np.ascontiguousarray(xp.transpose(2, 1, 0))
        # noiseT16[p, m, t, b] = noise[b, t, m*128+p]
        nT = np.ascontiguousarray(
            npad.reshape(B, T_LOC, RC, 128).transpose(3, 2, 1, 0))
        in_maps.append({
            "xT16": xT,
            "noiseT16": nT,
            "w16": w16,
            "win16": win16,
            "b32": b32,
        })
    return in_maps, w_eff.astype(np.float32)


def reference_np(x, w_in, b_rec, w_eff, noise, T=None):
    """Numpy reference for dev checks (f32)."""
    x = np.asarray(x, np.float32)
    if T is None:
        T = x.shape[1]
    z = np.einsum("bti,ir->btr", x[:, :T], np.asarray(w_in)) \
        + np.asarray(noise)[:, :T] + np.asarray(b_rec)
    h = np.zeros((x.shape[0], w_eff.shape[0]), np.float32)
    outs = []
    for t in range(T):
        pre = z[:, t] + h @ w_eff
        h = LEAK * h + ALPHA * np.maximum(pre, 0.0)
        outs.append(h.copy())
    return np.stack(outs, axis=1)


# ---------------------------------------------------------------------------
# harness entry point
# ---------------------------------------------------------------------------
_NC_CACHE = {}


def kernel(x, w_in, w_rec, b_rec, ei_mask, autapse_mask, noise):
    from concourse.bass_utils import run_bass_kernel_spmd

    x = np.asarray(x)
    T = x.shape[1]
    in_maps, _ = host_prep(x, w_in, w_rec, b_rec, ei_mask, autapse_mask, noise)
    if T not in _NC_CACHE:
        _NC_CACHE[T] = build_nc()
    nc = _NC_CACHE[T]
    res = run_bass_kernel_spmd(nc, in_maps, core_ids=list(range(N_CORES)))
    out = np.empty((x.shape[0], T, R), np.float32)
    for c in range(N_CORES):
        out[:, c * T_OUT:(c + 1) * T_OUT] = res.results[c]["out_c"]
    return out


# revision 21
# speedup vs baseline: 3.8038x; 1.2543x over previous
"""BioRNN Trainium2 kernel (dev module).

Sharding: time x8 (125-step output windows, full batch 64 per core).
The leak (0.8/step) makes the state forget: starting a window 100 steps
early from h=0 reproduces the true state to ~1e-5 rel, so the 8 time
shards run independently with a 100-step burn-in (core 0 pads z with
zeros, exact). Per core: T=225 steps, B=64 batch.

accum-q recurrence (fp16 weights/state, no per-step leak matmuls):
within a Q=32 step block (j = t % Q), psum holds q = 0.8^-j * p where
p_t = h_{t-1} @ w_eff. Then q_{t+1} = q_t + r'_t @ w_eff with
    r'_t = 0.8^-(j+1) * r_t = relu((q + 0.8^-j * z_t) * 0.25)   (DVE)
    h_t  = 0.8*h_{t-1} + 0.8^(j+1) * r'_t                       (DVE)
Every Q steps the bank is re-injected at true scale via ACT mul
(0.8^Q * q -> fp16) + identity matmuls with start=True. 16 W-matmuls
per step, zero leak matmuls in steady state.

Inputs arrive host-pre-transposed fp16 (noiseT[p, m, t, b], xT[p, t, b])
so the prepass is just: DMA chunk into the z ring, then 4 matmuls
(x @ w_in) + 4 DVE adds per 16-step chunk, emitted spread-out, one
chunk ~24 steps ahead of use.

Layouts (partition dim = r-chunk of 128; 4 chunks m=0..3):
  zring sbuf fp16 (128, RC*ZR*B)  ring of ZR=128 steps of z
  h16   sbuf fp16 (128, U*SUP)    ring of U=128 steps, col = t*SUP+m*B+b
  w16   sbuf fp16 (128, 4*512)    [p, k*512+m*128+c] = W[k*128+p, m*128+c]
  xT16  sbuf fp16 (128, T*B)      col = t*B + b  (n_in on partitions)
  psum: bank A = q cols m0|m1 (2*B), bank C = q cols m2|m3.

Step order: bank-A mms first (k01 gated by r'a, k23 by r'b), so RA_a of
the next step unblocks after 8 matmuls; bank-C mms run in the shadow.

Output: h16 ring chunks are DMA'd straight to DRAM as fp16 in their
native (p, t, m, b) layout (8-step chunks, round-robin across the four
DMA-capable queues); the host un-transposes to (b, t, r) f32. No PE
transposes, no ACT copies, half the output traffic.
"""

import numpy as np
from contextlib import ExitStack

import concourse.bass as bass
import concourse.mybir as mybir
import concourse.tile as tile
from concourse import bacc
from concourse import dve_ops
from concourse.dve_spec import (
    Spec, Src0, Src1, C0, C1, relu as _dve_relu_expr, lower,
)
from concourse.dve_uop import DveOpSpec
from concourse.masks import make_identity


def _register_dve(name, body, ref):
    """Register a custom DVE op (idempotent)."""
    for o in dve_ops.OPS:
        if o.name == name:
            return o
    opcode = max(dve_ops._SUB_OPCODE_FOR_NAME.values()) + 1
    assert opcode < 0x20
    dve_ops._SUB_OPCODE_FOR_NAME[name] = opcode
    spec = Spec(body=body, reference=ref)
    shas = {}
    for ver in ("v3", "v4"):
        s = DveOpSpec(name=name, opcode=opcode, uops=lower(spec, ver=ver),
                      rd1_en=True)
        shas[ver] = s.sha(ver)
    op = dve_ops.DveOp(name, spec, subdim=False, uops_sha=shas)
    dve_ops.OPS.append(op)
    dve_ops.CUSTOM_DVE_SPECS[name] = spec
    return op


def _f32(a):
    return a.astype(np.float32).reshape(a.shape[0], -1)


def _ref_relu_qz(in0, in1, c0, c1, c2):
    s = np.maximum(np.nan_to_num((_f32(in0) + _f32(in1) * c1) * c0,
                                 nan=0.0, posinf=np.inf, neginf=-np.inf), 0)
    return s.reshape(in0.shape)


def _ref_leak2(in0, in1, c0, c1, c2):
    return (_f32(in0) * c0 + _f32(in1) * c1).reshape(in0.shape)


RELU_QZ = _register_dve("RELU_QZ_BIO",
                        _dve_relu_expr((Src0 + Src1 * C1) * C0), _ref_relu_qz)
LEAK2 = _register_dve("LEAK2_BIO", Src0 * C0 + Src1 * C1, _ref_leak2)

F32 = mybir.dt.float32
F16 = mybir.dt.float16
AOP = mybir.AluOpType

R = 512          # n_rec
NIN = 128        # n_in
RC = 4           # r chunks (m and k)
N_CORES = 8
TSPLIT = 8       # time shards
B = 64           # batch per core (full batch)
SUP = RC * B     # cols per step supertile
T_FULL = 1000
T_OUT = T_FULL // TSPLIT  # output steps per core
BURN = 100                # burn-in steps (truncation err ~1e-5)
T_LOC = T_OUT + BURN      # local steps per core
OUT0 = BURN               # first local step that produces output
ALPHA = 0.2
LEAK = 1.0 - ALPHA
Q = 32                    # accum-q rescale block
ZR = 128                  # z ring steps
ZCH = 16                  # z chunk (DMA + zmm granularity)
ZLEAD = 32                # emit z chunk this many steps ahead


def build_nc(T=T_LOC, U=128, use_bacc=True):
    """Build the per-core Bass program. U = h-ring steps."""
    OBLK = 64
    assert U % (2 * OBLK) == 0
    nc = bacc.Bacc() if use_bacc else bass.Bass()

    # host-pre-transposed fp16 inputs
    xT_d = nc.dram_tensor("xT16", [NIN, T, B], F16, kind="ExternalInput").ap()
    nT_d = nc.dram_tensor("noiseT16", [128, RC, T, B], F16,
                          kind="ExternalInput").ap()
    w_d = nc.dram_tensor("w16", [R, R], F16, kind="ExternalInput").ap()
    wi_d = nc.dram_tensor("win16", [NIN, R], F16, kind="ExternalInput").ap()
    b_d = nc.dram_tensor("b32", [R], F32, kind="ExternalInput").ap()
    # raw h16 dump: [p, t_out, m*B+b] fp16; host un-transposes
    o_d = nc.dram_tensor("outT16", [128, T_OUT, SUP], F16,
                         kind="ExternalOutput").ap()

    with tile.TileContext(nc) as tc, ExitStack() as ctx:
        const = ctx.enter_context(tc.tile_pool(name="const", bufs=1))
        big = ctx.enter_context(tc.tile_pool(name="big", bufs=1))

        # ---- constants ----
        ident16 = const.tile([128, 128], F16)
        make_identity(nc, ident16[:, :])

        w16 = const.tile([128, RC * R], F16)
        nc.sync.dma_start(
            out=w16[:, :].rearrange("p (k m) -> p k m", m=R),
            in_=w_d.rearrange("(k p) m -> p k m", p=128),
        )
        win16 = const.tile([128, R], F16)
        nc.sync.dma_start(out=win16[:, :], in_=wi_d)
        b32 = const.tile([128, RC], F32)
        nc.sync.dma_start(out=b32[:, :], in_=b_d.rearrange("(m p) -> p m", p=128))

        # ---- big persistent buffers ----
        zring = big.tile([128, RC * ZR * B], F16)
        xT16 = big.tile([128, T * B], F16)
        h16 = big.tile([128, U * SUP], F16)
        nc.vector.memset(h16[:, (U - 1) * SUP:U * SUP], 0.0)

        zv = zring[:, :].rearrange("p (m t b) -> p m t b", t=ZR, b=B)
        hv = h16[:, :].rearrange("p (t m b) -> p t m b", m=RC, b=B)

        # xT16 loaded in two pieces (first small for fast start)
        for (t0, t1) in ((0, ZCH), (ZCH, T)):
            nc.sync.dma_start(out=xT16[:, t0 * B:t1 * B],
                              in_=xT_d[:, t0:t1, :])

        ps_z = ctx.enter_context(tc.tile_pool(name="psz", bufs=2, space="PSUM"))
        ZB = 8  # zmm steps per matmul (8*B = 512 moving cols)

        # round-robin the bulk DMAs over the three DMA-capable queues
        dmaq = [nc.gpsimd, nc.sync, nc.scalar]
        qi = [0]

        def next_q():
            qi[0] = (qi[0] + 1) % len(dmaq)
            return dmaq[qi[0]]

        def emit_z_chunk(t0):
            """DMA noise subchunks into ring (rotating queues) + z = x @
            w_in + b + noise."""
            t1 = min(t0 + ZCH, T)
            for z0 in range(t0, t1, ZB):
                nt = min(ZB, t1 - z0)
                rz = z0 % ZR
                next_q().dma_start(out=zv[:, :, rz:rz + nt, :],
                                   in_=nT_d[:, :, z0:z0 + nt, :])
                for m in range(RC):
                    zps = ps_z.tile([128, ZB * B], F32, tag="zps")
                    nc.tensor.matmul(
                        zps[:, :nt * B],
                        lhsT=win16[:, m * 128:(m + 1) * 128],
                        rhs=xT16[:, z0 * B:(z0 + nt) * B],
                        start=True, stop=True,
                    )
                    zsl = zv[:, m, rz:rz + nt, :]
                    nc.vector.scalar_tensor_tensor(
                        out=zsl,
                        in0=zps[:, :nt * B].rearrange("p (t b) -> p t b", b=B),
                        scalar=b32[:, m:m + 1], in1=zsl,
                        op0=AOP.add, op1=AOP.add,
                    )

        # ---- output dump chunks: <=8 steps, never crossing U-multiples ----
        ochunks = []
        a = OUT0
        while a < T:
            b_end = min(a + 8, T, ((a // U) + 1) * U)
            ochunks.append((a, b_end))
            a = b_end
        # emit chunk (a, e) at loop step t == e (reads ring rows a..e-1)
        def emit_out_chunk(ci):
            a, e = ochunks[ci]
            s0 = (a % U) * SUP
            next_q().dma_start(out=o_d[:, a - OUT0:e - OUT0, :],
                               in_=h16[:, s0:s0 + (e - a) * SUP]
                               .rearrange("p (t s) -> p t s", s=SUP))

        # ---- recurrence + interleaved output drain ----
        with tc.tile_pool(name="rp", bufs=2) as rp, \
             tc.tile_pool(name="sp", bufs=2) as sp, \
             tc.tile_pool(name="psA", bufs=1, space="PSUM") as ps_a, \
             tc.tile_pool(name="psC", bufs=1, space="PSUM") as ps_c:
            psA = ps_a.tile([128, 512], F32, name="psa", tag="psa")
            psC = ps_c.tile([128, 512], F32, name="psc", tag="psc")
            pvA = psA[:, :2 * B].rearrange("p (m c) -> p m c", c=B)
            pvC = psC[:, :2 * B].rearrange("p (m c) -> p m c", c=B)

            zero16 = const.tile([128, B], F16)
            nc.vector.memset(zero16[:, :], 0.0)

            def ps_of(m):
                ps = psA if m < 2 else psC
                return ps, (m % 2) * B

            # prime q = p_0 = 0
            for m in range(RC):
                ps, off = ps_of(m)
                nc.tensor.matmul(ps[:, off:off + B], lhsT=w16[:, 0:128],
                                 rhs=zero16[:, :], start=(m % 2 == 0),
                                 stop=(m % 2 == 1), skip_group_check=True)

            emit_z_chunk(0)
            emit_z_chunk(ZCH)
            prev_r = None
            for t in range(T):
                if (t + ZLEAD) % ZCH == 0 and 2 * ZCH <= t + ZLEAD < T:
                    emit_z_chunk(t + ZLEAD)
                rd = ((t - 1) % U) * SUP
                wr = (t % U) * SUP
                rbig = rp.tile([128, SUP], F16, tag="rbig")
                jp = t % Q          # frame of q after this iteration's mms
                jn = (t + 1) % Q    # frame after the next iteration's mms
                if t > 0:
                    if jp == 0:
                        # restart: re-inject q at true scale (q := 0.8^Q * q)
                        s16a = sp.tile([128, 2 * B], F16, tag="s16a")
                        s16b = sp.tile([128, 2 * B], F16, tag="s16b")
                        nc.scalar.mul(out=s16a[:, :], in_=psA[:, :2 * B],
                                      mul=float(LEAK ** Q))
                        nc.scalar.mul(out=s16b[:, :], in_=psC[:, :2 * B],
                                      mul=float(LEAK ** Q))
                        for m in range(RC):
                            ps, off = ps_of(m)
                            src = s16a if m < 2 else s16b
                            nc.tensor.matmul(
                                ps[:, off:off + B], lhsT=ident16[:, :],
                                rhs=src[:, (m % 2) * B:(m % 2 + 1) * B],
                                start=(m % 2 == 0), stop=False,
                                skip_group_check=True)

                    def kmm(m, k, stop=False):
                        ps, off = ps_of(m)
                        return nc.tensor.matmul(
                            ps[:, off:off + B],
                            lhsT=w16[:, k * R + m * 128:k * R + (m + 1) * 128],
                            rhs=prev_r[:, k * B:(k + 1) * B],
                            start=False, stop=stop, skip_group_check=True)

                    # bank A (m01) first: k01 (gated by r'a) then k23 (r'b)
                    kmm(0, 0); kmm(1, 0); kmm(0, 1); kmm(1, 1)
                    kmm(0, 2); kmm(1, 2); kmm(0, 3); kmm(1, 3, stop=True)
                    # bank C (m23) in the shadow
                    kmm(2, 0); kmm(3, 0); kmm(2, 1); kmm(3, 1)
                    kmm(2, 2); kmm(3, 2); kmm(2, 3); kmm(3, 3, stop=True)

                # r' = relu((q + 0.8^-jp * z) * 0.2*0.8^(jp-jn))   (DVE)
                s0 = float(ALPHA * LEAK ** (jp - jn))
                s1 = float(LEAK ** (-jp))
                zt = t % ZR
                nc.vector._custom_dve(
                    RELU_QZ,
                    out=rbig[:, :2 * B].rearrange("p (m c) -> p m c", c=B),
                    in0=pvA[:, 0:2, 0:B], in1=zv[:, 0:2, zt, :],
                    s0=s0, s1=s1)
                nc.vector._custom_dve(
                    RELU_QZ,
                    out=rbig[:, 2 * B:].rearrange("p (m c) -> p m c", c=B),
                    in0=pvC[:, 0:2, 0:B], in1=zv[:, 2:4, zt, :],
                    s0=s0, s1=s1)
                # h output: h_t = 0.8*h_{t-1} + 0.8^jn * r'  (off critical path)
                nc.vector._custom_dve(
                    LEAK2,
                    out=h16[:, wr:wr + SUP], in0=h16[:, rd:rd + SUP],
                    in1=rbig[:, :], s0=float(LEAK), s1=float(LEAK ** jn))
                prev_r = rbig
                for ci, (a, e) in enumerate(ochunks):
                    if t == e:
                        emit_out_chunk(ci)
            for ci, (a, e) in enumerate(ochunks):
                if e >= T:
                    emit_out_chunk(ci)

    if use_bacc:
        nc.compile()
    return nc


def host_prep(x, w_in, w_rec, b_rec, ei_mask, autapse_mask, noise):
    """Host-side weight prep + time shard + fp16 layout transpose."""
    ei = np.diagonal(np.asarray(ei_mask)).astype(np.float32)
    w_eff = ei[:, None] * (np.asarray(w_rec) * np.asarray(autapse_mask))
    w16 = w_eff.astype(np.float16)
    win16 = np.asarray(w_in).astype(np.float16)
    b32 = np.asarray(b_rec).astype(np.float32)
    x = np.asarray(x, dtype=np.float32)
    noise = np.asarray(noise, dtype=np.float32)
    in_maps = []
    for c in range(N_CORES):
        t0 = c * T_OUT - BURN
        xp = np.zeros((B, T_LOC, NIN), np.float16)
        npad = np.zeros((B, T_LOC, R), np.float16)
        s = max(t0, 0)
        off = s - t0
        xp[:, off:] = x[:, s:t0 + T_LOC]
        npad[:, off:] = noise[:, s:t0 + T_LOC]
        # xT16[p, t, b] = x[b, t, p]
        xT = np.ascontiguousarray(xp.transpose(2, 1, 0))
        # noiseT16[p, m, t, b] = noise[b, t, m*128+p]
        nT = np.ascontiguousarray(
            npad.reshape(B, T_LOC, RC, 128).transpose(3, 2, 1, 0))
        in_maps.append({
            "xT16": xT,
            "noiseT16": nT,
            "w16": w16,
            "win16": win16,
            "b32": b32,
        })
    return in_maps, w_eff.astype(np.float32)


def reference_np(x, w_in, b_rec, w_eff, noise, T=None):
    """Numpy reference for dev checks (f32)."""
    x = np.asarray(x, np.float32)
    if T is None:
        T = x.shape[1]
    z = np.einsum("bti,ir->btr", x[:, :T], np.asarray(w_in)) \
        + np.asarray(noise)[:, :T] + np.asarray(b_rec)
    h = np.zeros((x.shape[0], w_eff.shape[0]), np.float32)
    outs = []
    for t in range(T):
        pre = z[:, t] + h @ w_eff
        h = LEAK * h + ALPHA * np.maximum(pre, 0.0)
        outs.append(h.copy())
    return np.stack(outs, axis=1)


# ---------------------------------------------------------------------------
# harness entry point
# ---------------------------------------------------------------------------
_NC_CACHE = {}


def kernel(x, w_in, w_rec, b_rec, ei_mask, autapse_mask, noise):
    from concourse.bass_utils import run_bass_kernel_spmd

    x = np.asarray(x)
    T = x.shape[1]
    in_maps, _ = host_prep(x, w_in, w_rec, b_rec, ei_mask, autapse_mask, noise)
    if T not in _NC_CACHE:
        _NC_CACHE[T] = build_nc()
    nc = _NC_CACHE[T]
    res = run_bass_kernel_spmd(nc, in_maps, core_ids=list(range(N_CORES)))
    out = np.empty((x.shape[0], T, R), np.float32)
    for c in range(N_CORES):
        # dump[p, t, m*B+b] = h[b, t, m*128+p]
        dump = res.results[c]["outT16"]
        out[:, c * T_OUT:(c + 1) * T_OUT] = (
            dump.reshape(128, T_OUT, RC, B).transpose(3, 1, 2, 0)
            .reshape(B, T_OUT, R).astype(np.float32))
    return out


# revision 23
# speedup vs baseline: 4.2244x; 1.1106x over previous
"""BioRNN Trainium2 kernel (dev module).

Sharding: time x8 (125-step output windows, full batch 64 per core).
The leak (0.8/step) makes the state forget: starting a window 100 steps
early from h=0 reproduces the true state to ~1e-5 rel, so the 8 time
shards run independently with a 100-step burn-in (core 0 pads inputs
with zeros, exact). Per core: T=225 steps, B=64 batch.

delta-injection accum-q recurrence (fp16, no per-step leak matmuls, no
per-step DVE z-add). psum holds q = 0.8^-j * p'_t within a Q=32 block
(j = t % Q), where p'_t = z_t + h_{t-1} @ w_eff is the full pre-
activation. Since p'_{t+1} = 0.8 p'_t + r_t @ w_eff + delta_{t+1} with
delta_t = z_t - 0.8 z_{t-1}, each step accumulates into psum:
    8 delta matmuls:  dxT_t @ w_in (4) + identity @ dnT_t (4)
    16 W matmuls:     r'_t @ w_eff
where dxT/dnT are HOST-precomputed deltas, pre-scaled by 0.8^-j(t),
fp16, in transposed layout. Then on DVE:
    r'_t = relu(q * 0.2*0.8^(jp-jn))            (RELU_SC, 1 input)
    h_t  = 0.8*h_{t-1} + 0.8^jn * r'_t          (LEAK2)
Every Q steps the bank is re-injected at true scale via ACT mul
(0.8^Q * q -> fp16) + identity matmuls with start=True.

PE order per step keeps the RA_a chain short: [injA | k01m01] (gated by
r'a) -> k23m01 (gated by r'b, stop A) -> [injC | k01m23 | k23m23]
(bank C in the shadow). RA_b-gated work never sits ahead of RA_a-gated
work in the in-order PE queue.

Output: h16 ring chunks are DMA'd straight to DRAM as fp16 in their
native (p, t, m, b) layout (8-step chunks, round-robin across the three
DMA-capable queues); the host un-transposes to (b, t, r) f32.
"""

import numpy as np
from contextlib import ExitStack

import concourse.bass as bass
import concourse.mybir as mybir
import concourse.tile as tile
from concourse import bacc
from concourse import dve_ops
from concourse.dve_spec import (
    Spec, Src0, Src1, C0, C1, relu as _dve_relu_expr, lower,
)
from concourse.dve_uop import DveOpSpec
from concourse.masks import make_identity


def _register_dve(name, body, ref, rd1=True):
    """Register a custom DVE op (idempotent)."""
    for o in dve_ops.OPS:
        if o.name == name:
            return o
    opcode = max(dve_ops._SUB_OPCODE_FOR_NAME.values()) + 1
    assert opcode < 0x20
    dve_ops._SUB_OPCODE_FOR_NAME[name] = opcode
    spec = Spec(body=body, reference=ref)
    shas = {}
    for ver in ("v3", "v4"):
        s = DveOpSpec(name=name, opcode=opcode, uops=lower(spec, ver=ver),
                      rd1_en=rd1)
        shas[ver] = s.sha(ver)
    op = dve_ops.DveOp(name, spec, subdim=False, uops_sha=shas)
    dve_ops.OPS.append(op)
    dve_ops.CUSTOM_DVE_SPECS[name] = spec
    return op


def _f32(a):
    return a.astype(np.float32).reshape(a.shape[0], -1)


def _ref_relu_sc(in0, in1, c0, c1, c2):
    s = np.maximum(np.nan_to_num(_f32(in0) * c0,
                                 nan=0.0, posinf=np.inf, neginf=-np.inf), 0)
    return s.reshape(in0.shape)


def _ref_leak2(in0, in1, c0, c1, c2):
    return (_f32(in0) * c0 + _f32(in1) * c1).reshape(in0.shape)


RELU_SC = _register_dve("RELU_SC_BIO", _dve_relu_expr(Src0 * C0),
                        _ref_relu_sc, rd1=False)
LEAK2 = _register_dve("LEAK2_BIO", Src0 * C0 + Src1 * C1, _ref_leak2)

F32 = mybir.dt.float32
F16 = mybir.dt.float16
AOP = mybir.AluOpType

R = 512          # n_rec
NIN = 128        # n_in
RC = 4           # r chunks (m and k)
N_CORES = 8
TSPLIT = 8       # time shards
B = 64           # batch per core (full batch)
SUP = RC * B     # cols per step supertile
T_FULL = 1000
T_OUT = T_FULL // TSPLIT  # output steps per core
BURN = 100                # burn-in steps (truncation err ~1e-5)
T_LOC = T_OUT + BURN      # local steps per core
OUT0 = BURN               # first local step that produces output
ALPHA = 0.2
LEAK = 1.0 - ALPHA
Q = 32                    # accum-q rescale block
ZR = 128                  # delta ring steps
ZCH = 16                  # delta chunk (DMA granularity)
ZLEAD = 96                # chunks are DMA'd this many steps ahead


def build_nc(T=T_LOC, U=128, use_bacc=True):
    """Build the per-core Bass program. U = h-ring steps."""
    nc = bacc.Bacc() if use_bacc else bass.Bass()

    # host-precomputed pre-scaled deltas, fp16, transposed layouts
    dxT_d = nc.dram_tensor("dxT16", [NIN, T, B], F16, kind="ExternalInput").ap()
    dnT_d = nc.dram_tensor("dnT16", [128, RC, T, B], F16,
                           kind="ExternalInput").ap()
    w_d = nc.dram_tensor("w16", [R, R], F16, kind="ExternalInput").ap()
    wi_d = nc.dram_tensor("win16", [NIN, R], F16, kind="ExternalInput").ap()
    # raw h16 dump: [p, t_out, m*B+b] fp16; host un-transposes
    o_d = nc.dram_tensor("outT16", [128, T_OUT, SUP], F16,
                         kind="ExternalOutput").ap()

    with tile.TileContext(nc) as tc, ExitStack() as ctx:
        const = ctx.enter_context(tc.tile_pool(name="const", bufs=1))
        big = ctx.enter_context(tc.tile_pool(name="big", bufs=1))

        # ---- constants ----
        ident16 = const.tile([128, 128], F16)
        make_identity(nc, ident16[:, :])

        w16 = const.tile([128, RC * R], F16)
        nc.sync.dma_start(
            out=w16[:, :].rearrange("p (k m) -> p k m", m=R),
            in_=w_d.rearrange("(k p) m -> p k m", p=128),
        )
        win16 = const.tile([128, R], F16)
        nc.sync.dma_start(out=win16[:, :], in_=wi_d)

        # ---- big persistent buffers ----
        dring = big.tile([128, RC * ZR * B], F16)   # delta-noise ring
        xT16 = big.tile([128, T * B], F16)          # delta-x, full resident
        h16 = big.tile([128, U * SUP], F16)
        nc.vector.memset(h16[:, (U - 1) * SUP:U * SUP], 0.0)

        dv = dring[:, :].rearrange("p (m t b) -> p m t b", t=ZR, b=B)

        # delta-x: small first piece for fast start, bulk on the ACT queue
        nc.scalar.dma_start(out=xT16[:, :ZCH * B], in_=dxT_d[:, :ZCH, :])
        nc.scalar.dma_start(out=xT16[:, ZCH * B:], in_=dxT_d[:, ZCH:, :])

        # round-robin the bulk DMAs over the three DMA-capable queues
        dmaq = [nc.gpsimd, nc.sync, nc.scalar]
        qi = [0]

        def next_q():
            qi[0] = (qi[0] + 1) % len(dmaq)
            return dmaq[qi[0]]

        def emit_dn_chunk(t0):
            t1 = min(t0 + ZCH, T)
            for z0 in range(t0, t1, 8):
                nt = min(8, t1 - z0)
                rz = z0 % ZR
                next_q().dma_start(out=dv[:, :, rz:rz + nt, :],
                                   in_=dnT_d[:, :, z0:z0 + nt, :])

        # ---- output dump chunks: <=8 steps, never crossing U-multiples ----
        ochunks = []
        a = OUT0
        while a < T:
            e = min(a + 8, T, ((a // U) + 1) * U)
            ochunks.append((a, e))
            a = e

        def emit_out_chunk(ci):
            a, e = ochunks[ci]
            s0 = (a % U) * SUP
            next_q().dma_start(out=o_d[:, a - OUT0:e - OUT0, :],
                               in_=h16[:, s0:s0 + (e - a) * SUP]
                               .rearrange("p (t s) -> p t s", s=SUP))

        # ---- recurrence ----
        with tc.tile_pool(name="rp", bufs=2) as rp, \
             tc.tile_pool(name="sp", bufs=2) as sp, \
             tc.tile_pool(name="psA", bufs=1, space="PSUM") as ps_a, \
             tc.tile_pool(name="psC", bufs=1, space="PSUM") as ps_c:
            psA = ps_a.tile([128, 512], F32, name="psa", tag="psa")
            psC = ps_c.tile([128, 512], F32, name="psc", tag="psc")
            pvA = psA[:, :2 * B].rearrange("p (m c) -> p m c", c=B)
            pvC = psC[:, :2 * B].rearrange("p (m c) -> p m c", c=B)

            zero16 = const.tile([128, B], F16)
            nc.vector.memset(zero16[:, :], 0.0)

            def ps_of(m):
                ps = psA if m < 2 else psC
                return ps, (m % 2) * B

            def dinj(m, stop=False):
                """delta injections for chunk m: dx@w_in then ident@dn."""
                ps, off = ps_of(m)
                nc.tensor.matmul(
                    ps[:, off:off + B],
                    lhsT=win16[:, m * 128:(m + 1) * 128],
                    rhs=xT16[:, t * B:(t + 1) * B],
                    start=False, stop=False, skip_group_check=True)
                nc.tensor.matmul(
                    ps[:, off:off + B], lhsT=ident16[:, :],
                    rhs=dv[:, m, t % ZR, :],
                    start=False, stop=stop, skip_group_check=True)

            # prime q = 0, then inject delta_0 (= z_0)
            for m in range(RC):
                ps, off = ps_of(m)
                nc.tensor.matmul(ps[:, off:off + B], lhsT=w16[:, 0:128],
                                 rhs=zero16[:, :], start=(m % 2 == 0),
                                 stop=False, skip_group_check=True)

            for c0 in range(0, ZLEAD + ZCH, ZCH):
                emit_dn_chunk(c0)
            prev_r = None
            for t in range(T):
                if (t + ZLEAD) % ZCH == 0 and ZLEAD + ZCH <= t + ZLEAD < T:
                    emit_dn_chunk(t + ZLEAD)
                rd = ((t - 1) % U) * SUP
                wr = (t % U) * SUP
                rbig = rp.tile([128, SUP], F16, tag="rbig")
                jp = t % Q          # frame of q after this iteration's mms
                jn = (t + 1) % Q    # frame after the next iteration's mms
                if t == 0:
                    for m in range(RC):
                        dinj(m, stop=(m % 2 == 1))
                else:
                    if jp == 0:
                        # restart: re-inject q at true scale (q := 0.8^Q * q)
                        s16a = sp.tile([128, 2 * B], F16, tag="s16a")
                        s16b = sp.tile([128, 2 * B], F16, tag="s16b")
                        nc.scalar.mul(out=s16a[:, :], in_=psA[:, :2 * B],
                                      mul=float(LEAK ** Q))
                        nc.scalar.mul(out=s16b[:, :], in_=psC[:, :2 * B],
                                      mul=float(LEAK ** Q))
                        for m in range(RC):
                            ps, off = ps_of(m)
                            src = s16a if m < 2 else s16b
                            nc.tensor.matmul(
                                ps[:, off:off + B], lhsT=ident16[:, :],
                                rhs=src[:, (m % 2) * B:(m % 2 + 1) * B],
                                start=(m % 2 == 0), stop=False,
                                skip_group_check=True)

                    def kmm(m, k, stop=False):
                        ps, off = ps_of(m)
                        return nc.tensor.matmul(
                            ps[:, off:off + B],
                            lhsT=w16[:, k * R + m * 128:k * R + (m + 1) * 128],
                            rhs=prev_r[:, k * B:(k + 1) * B],
                            start=False, stop=stop, skip_group_check=True)

                    # bank A: injections + k01 (gated by r'a), then k23 (r'b)
                    dinj(0); dinj(1)
                    kmm(0, 0); kmm(1, 0); kmm(0, 1); kmm(1, 1)
                    kmm(0, 2); kmm(1, 2); kmm(0, 3); kmm(1, 3, stop=True)
                    # bank C in the shadow (all its gates are now resolved)
                    dinj(2); dinj(3)
                    kmm(2, 0); kmm(3, 0); kmm(2, 1); kmm(3, 1)
                    kmm(2, 2); kmm(3, 2); kmm(2, 3); kmm(3, 3, stop=True)

                # r' = relu(q * 0.2*0.8^(jp-jn))   (DVE, psum in only)
                s0 = float(ALPHA * LEAK ** (jp - jn))
                nc.vector._custom_dve(
                    RELU_SC,
                    out=rbig[:, :2 * B].rearrange("p (m c) -> p m c", c=B),
                    in0=pvA[:, 0:2, 0:B], s0=s0)
                nc.vector._custom_dve(
                    RELU_SC,
                    out=rbig[:, 2 * B:].rearrange("p (m c) -> p m c", c=B),
                    in0=pvC[:, 0:2, 0:B], s0=s0)
                # h output: h_t = 0.8*h_{t-1} + 0.8^jn * r'  (off critical path)
                nc.vector._custom_dve(
                    LEAK2,
                    out=h16[:, wr:wr + SUP], in0=h16[:, rd:rd + SUP],
                    in1=rbig[:, :], s0=float(LEAK), s1=float(LEAK ** jn))
                prev_r = rbig
                for ci, (a, e) in enumerate(ochunks):
                    if t == e:
                        emit_out_chunk(ci)
            for ci, (a, e) in enumerate(ochunks):
                if e >= T:
                    emit_out_chunk(ci)

    if use_bacc:
        nc.compile()
    return nc


def host_prep(x, w_in, w_rec, b_rec, ei_mask, autapse_mask, noise):
    """Host-side weight prep + time shard + pre-scaled fp16 delta inputs.

    delta_t = z_t - 0.8*z_{t-1} split into x and noise parts, scaled by
    0.8^-(t % Q) to match the psum accumulation frame. b_rec is folded
    into the noise part (constant offset of z).
    """
    ei = np.diagonal(np.asarray(ei_mask)).astype(np.float32)
    w_eff = ei[:, None] * (np.asarray(w_rec) * np.asarray(autapse_mask))
    w16 = w_eff.astype(np.float16)
    win16 = np.asarray(w_in).astype(np.float16)
    x = np.asarray(x, dtype=np.float32)
    nb = np.asarray(noise, dtype=np.float32) + np.asarray(b_rec, np.float32)
    jscale = (LEAK ** -(np.arange(T_LOC) % Q)).astype(np.float32)
    in_maps = []
    for c in range(N_CORES):
        t0 = c * T_OUT - BURN
        xp = np.zeros((B, T_LOC, NIN), np.float32)
        npad = np.zeros((B, T_LOC, R), np.float32)
        s = max(t0, 0)
        off = s - t0
        xp[:, off:] = x[:, s:t0 + T_LOC]
        npad[:, off:] = nb[:, s:t0 + T_LOC]
        dx = xp.copy()
        dx[:, 1:] -= LEAK * xp[:, :-1]
        dn = npad.copy()
        dn[:, 1:] -= LEAK * npad[:, :-1]
        dx *= jscale[None, :, None]
        dn *= jscale[None, :, None]
        dxT = np.ascontiguousarray(
            dx.astype(np.float16).transpose(2, 1, 0))
        dnT = np.ascontiguousarray(
            dn.astype(np.float16).reshape(B, T_LOC, RC, 128)
            .transpose(3, 2, 1, 0))
        in_maps.append({
            "dxT16": dxT,
            "dnT16": dnT,
            "w16": w16,
            "win16": win16,
        })
    return in_maps, w_eff.astype(np.float32)


def reference_np(x, w_in, b_rec, w_eff, noise, T=None):
    """Numpy reference for dev checks (f32)."""
    x = np.asarray(x, np.float32)
    if T is None:
        T = x.shape[1]
    z = np.einsum("bti,ir->btr", x[:, :T], np.asarray(w_in)) \
        + np.asarray(noise)[:, :T] + np.asarray(b_rec)
    h = np.zeros((x.shape[0], w_eff.shape[0]), np.float32)
    outs = []
    for t in range(T):
        pre = z[:, t] + h @ w_eff
        h = LEAK * h + ALPHA * np.maximum(pre, 0.0)
        outs.append(h.copy())
    return np.stack(outs, axis=1)


# ---------------------------------------------------------------------------
# harness entry point
# ---------------------------------------------------------------------------
_NC_CACHE = {}


def kernel(x, w_in, w_rec, b_rec, ei_mask, autapse_mask, noise):
    from concourse.bass_utils import run_bass_kernel_spmd

    x = np.asarray(x)
    T = x.shape[1]
    in_maps, _ = host_prep(x, w_in, w_rec, b_rec, ei_mask, autapse_mask, noise)
    if T not in _NC_CACHE:
        _NC_CACHE[T] = build_nc()
    nc = _NC_CACHE[T]
    res = run_bass_kernel_spmd(nc, in_maps, core_ids=list(range(N_CORES)))
    out = np.empty((x.shape[0], T, R), np.float32)
    for c in range(N_CORES):
        # dump[p, t, m*B+b] = h[b, t, m*128+p]
        dump = res.results[c]["outT16"]
        out[:, c * T_OUT:(c + 1) * T_OUT] = (
            dump.reshape(128, T_OUT, RC, B).transpose(3, 1, 2, 0)
            .reshape(B, T_OUT, R).astype(np.float32))
    return out


# revision 28
# speedup vs baseline: 4.3043x; 1.0189x over previous
"""BioRNN Trainium2 kernel (dev module).

Sharding: time x8 (125-step output windows, full batch 64 per core).
The leak (0.8/step) makes the state forget: starting a window 100 steps
early from h=0 reproduces the true state to ~1e-5 rel, so the 8 time
shards run independently with a 100-step burn-in (core 0 pads inputs
with zeros, exact). Per core: T=225 steps, B=64 batch.

delta-injection accum-q recurrence (fp16, no per-step leak matmuls, no
per-step DVE z-add). psum holds q = 0.8^-j * p'_t within a Q=32 block
(j = t % Q), where p'_t = z_t + h_{t-1} @ w_eff is the full pre-
activation. Since p'_{t+1} = 0.8 p'_t + r_t @ w_eff + delta_{t+1} with
delta_t = z_t - 0.8 z_{t-1}, each step accumulates into psum:
    8 delta matmuls:  dxT_t @ w_in (4) + identity @ dnT_t (4)
    16 W matmuls:     r'_t @ w_eff
where dxT/dnT are HOST-precomputed deltas, pre-scaled by 0.8^-j(t),
fp16, in transposed layout. Then on DVE:
    r'_t = relu(q * 0.2*0.8^(jp-jn))            (RELU_SC, 1 input)
    h_t  = 0.8*h_{t-1} + 0.8^jn * r'_t          (LEAK2)
Every Q steps the bank is re-injected at true scale via ACT mul
(0.8^Q * q -> fp16) + identity matmuls with start=True.

PE order per step keeps the RA_a chain short: [injA | k01m01] (gated by
r'a) -> k23m01 (gated by r'b, stop A) -> [injC | k01m23 | k23m23]
(bank C in the shadow). RA_b-gated work never sits ahead of RA_a-gated
work in the in-order PE queue.

Output: h16 ring chunks are DMA'd straight to DRAM as fp16 in their
native (p, t, m, b) layout (8-step chunks, round-robin across the three
DMA-capable queues); the host un-transposes to (b, t, r) f32.
"""

import numpy as np
from contextlib import ExitStack

import concourse.bass as bass
import concourse.mybir as mybir
import concourse.tile as tile
from concourse import bacc
from concourse import dve_ops
from concourse.dve_spec import (
    Spec, Src0, Src1, C0, C1, relu as _dve_relu_expr, lower,
)
from concourse.dve_uop import DveOpSpec
from concourse.masks import make_identity


def _register_dve(name, body, ref, rd1=True):
    """Register a custom DVE op (idempotent)."""
    for o in dve_ops.OPS:
        if o.name == name:
            return o
    opcode = max(dve_ops._SUB_OPCODE_FOR_NAME.values()) + 1
    assert opcode < 0x20
    dve_ops._SUB_OPCODE_FOR_NAME[name] = opcode
    spec = Spec(body=body, reference=ref)
    shas = {}
    for ver in ("v3", "v4"):
        s = DveOpSpec(name=name, opcode=opcode, uops=lower(spec, ver=ver),
                      rd1_en=rd1)
        shas[ver] = s.sha(ver)
    op = dve_ops.DveOp(name, spec, subdim=False, uops_sha=shas)
    dve_ops.OPS.append(op)
    dve_ops.CUSTOM_DVE_SPECS[name] = spec
    return op


def _f32(a):
    return a.astype(np.float32).reshape(a.shape[0], -1)


def _ref_relu_sc(in0, in1, c0, c1, c2):
    s = np.maximum(np.nan_to_num(_f32(in0) * c0,
                                 nan=0.0, posinf=np.inf, neginf=-np.inf), 0)
    return s.reshape(in0.shape)


def _ref_leak2(in0, in1, c0, c1, c2):
    return (_f32(in0) * c0 + _f32(in1) * c1).reshape(in0.shape)


RELU_SC = _register_dve("RELU_SC_BIO", _dve_relu_expr(Src0 * C0),
                        _ref_relu_sc, rd1=False)
LEAK2 = _register_dve("LEAK2_BIO", Src0 * C0 + Src1 * C1, _ref_leak2)

F32 = mybir.dt.float32
F16 = mybir.dt.float16
AOP = mybir.AluOpType

R = 512          # n_rec
NIN = 128        # n_in
RC = 4           # r chunks (m and k)
N_CORES = 8
TSPLIT = 8       # time shards
B = 64           # batch per core (full batch)
SUP = RC * B     # cols per step supertile
T_FULL = 1000
T_OUT = T_FULL // TSPLIT  # output steps per core
BURN = 100                # burn-in steps (truncation err ~1e-5)
T_LOC = T_OUT + BURN      # local steps per core
OUT0 = BURN               # first local step that produces output
ALPHA = 0.2
LEAK = 1.0 - ALPHA
Q = 32                    # accum-q rescale block
ZR = 128                  # delta ring steps
ZCH = 16                  # delta chunk (DMA granularity)
ZLEAD = 96                # chunks are DMA'd this many steps ahead


def build_nc(T=T_LOC, U=128, use_bacc=True):
    """Build the per-core Bass program. U = h-ring steps."""
    nc = bacc.Bacc() if use_bacc else bass.Bass()

    # host-precomputed pre-scaled deltas, fp16, transposed layouts
    dxT_d = nc.dram_tensor("dxT16", [NIN, T, B], F16, kind="ExternalInput").ap()
    dnT_d = nc.dram_tensor("dnT16", [128, RC, T, B], F16,
                           kind="ExternalInput").ap()
    w_d = nc.dram_tensor("w16", [R, R], F16, kind="ExternalInput").ap()
    wi_d = nc.dram_tensor("win16", [NIN, R], F16, kind="ExternalInput").ap()
    # raw h16 dump: [p, t_out, m*B+b] fp16; host un-transposes
    o_d = nc.dram_tensor("outT16", [128, T_OUT, SUP], F16,
                         kind="ExternalOutput").ap()

    with tile.TileContext(nc) as tc, ExitStack() as ctx:
        const = ctx.enter_context(tc.tile_pool(name="const", bufs=1))
        big = ctx.enter_context(tc.tile_pool(name="big", bufs=1))

        # ---- constants ----
        ident16 = const.tile([128, 128], F16)
        make_identity(nc, ident16[:, :])

        # ---- big persistent buffers ----
        dring = big.tile([128, RC * ZR * B], F16)   # delta-noise ring
        xT16 = big.tile([128, T * B], F16)          # delta-x, full resident
        h16 = big.tile([128, U * SUP], F16)
        nc.vector.memset(h16[:, (U - 1) * SUP:U * SUP], 0.0)

        dv = dring[:, :].rearrange("p (m t b) -> p m t b", t=ZR, b=B)

        # round-robin the bulk DMAs over the three DMA-capable queues
        dmaq = [nc.gpsimd, nc.sync, nc.scalar]
        qi = [0]

        def next_q():
            qi[0] = (qi[0] + 1) % len(dmaq)
            return dmaq[qi[0]]

        def emit_dn_chunk(t0, q=None):
            t1 = min(t0 + ZCH, T)
            for z0 in range(t0, t1, 8):
                nt = min(8, t1 - z0)
                rz = z0 % ZR
                (q or next_q()).dma_start(out=dv[:, :, rz:rz + nt, :],
                                          in_=dnT_d[:, :, z0:z0 + nt, :])

        # startup order: what step 0/1 needs first, spread over queues
        nc.scalar.dma_start(out=xT16[:, :ZCH * B], in_=dxT_d[:, :ZCH, :])
        emit_dn_chunk(0, q=nc.gpsimd)
        win16 = const.tile([128, R], F16)
        nc.scalar.dma_start(out=win16[:, :], in_=wi_d)
        w16 = const.tile([128, RC * R], F16)
        for k in range(RC):
            nc.sync.dma_start(out=w16[:, k * R:(k + 1) * R],
                              in_=w_d[k * 128:(k + 1) * 128, :])
        emit_dn_chunk(ZCH, q=nc.gpsimd)
        # delta-x bulk on the ACT queue
        nc.scalar.dma_start(out=xT16[:, ZCH * B:], in_=dxT_d[:, ZCH:, :])

        # ---- output dump chunks: <=8 steps, never crossing U-multiples ----
        ochunks = []
        a = OUT0
        while a < T:
            e = min(a + 8, T, ((a // U) + 1) * U)
            ochunks.append((a, e))
            a = e

        def emit_out_chunk(ci):
            a, e = ochunks[ci]
            s0 = (a % U) * SUP
            next_q().dma_start(out=o_d[:, a - OUT0:e - OUT0, :],
                               in_=h16[:, s0:s0 + (e - a) * SUP]
                               .rearrange("p (t s) -> p t s", s=SUP))

        # ---- recurrence ----
        with tc.tile_pool(name="rp", bufs=2) as rp, \
             tc.tile_pool(name="sp", bufs=2) as sp, \
             tc.tile_pool(name="psA", bufs=1, space="PSUM") as ps_a, \
             tc.tile_pool(name="psC", bufs=1, space="PSUM") as ps_c:
            psA = ps_a.tile([128, 512], F32, name="psa", tag="psa")
            psC = ps_c.tile([128, 512], F32, name="psc", tag="psc")
            pvA = psA[:, :2 * B].rearrange("p (m c) -> p m c", c=B)
            pvC = psC[:, :2 * B].rearrange("p (m c) -> p m c", c=B)

            zero16 = const.tile([128, B], F16)
            nc.vector.memset(zero16[:, :], 0.0)

            def ps_of(m):
                ps = psA if m < 2 else psC
                return ps, (m % 2) * B

            def dinj(m, stop=False):
                """delta injections for chunk m: dx@w_in then ident@dn."""
                ps, off = ps_of(m)
                nc.tensor.matmul(
                    ps[:, off:off + B],
                    lhsT=win16[:, m * 128:(m + 1) * 128],
                    rhs=xT16[:, t * B:(t + 1) * B],
                    start=False, stop=False, skip_group_check=True)
                nc.tensor.matmul(
                    ps[:, off:off + B], lhsT=ident16[:, :],
                    rhs=dv[:, m, t % ZR, :],
                    start=False, stop=stop, skip_group_check=True)

            # prime q = 0, then inject delta_0 (= z_0)
            for m in range(RC):
                ps, off = ps_of(m)
                nc.tensor.matmul(ps[:, off:off + B], lhsT=ident16[:, :],
                                 rhs=zero16[:, :], start=(m % 2 == 0),
                                 stop=False, skip_group_check=True)

            for c0 in range(2 * ZCH, ZLEAD + ZCH, ZCH):
                emit_dn_chunk(c0)
            prev_r = None
            for t in range(T):
                if (t + ZLEAD) % ZCH == 0 and ZLEAD + ZCH <= t + ZLEAD < T:
                    emit_dn_chunk(t + ZLEAD)
                rd = ((t - 1) % U) * SUP
                wr = (t % U) * SUP
                rbig = rp.tile([128, SUP], F16, tag="rbig")
                jp = t % Q          # frame of q after this iteration's mms
                jn = (t + 1) % Q    # frame after the next iteration's mms
                if t == 0:
                    for m in range(RC):
                        dinj(m, stop=(m % 2 == 1))
                else:
                    if jp == 0:
                        # restart: re-inject q at true scale (q := 0.8^Q * q)
                        s16a = sp.tile([128, 2 * B], F16, tag="s16a")
                        s16b = sp.tile([128, 2 * B], F16, tag="s16b")
                        nc.scalar.mul(out=s16a[:, :], in_=psA[:, :2 * B],
                                      mul=float(LEAK ** Q))
                        nc.scalar.mul(out=s16b[:, :], in_=psC[:, :2 * B],
                                      mul=float(LEAK ** Q))
                        for m in range(RC):
                            ps, off = ps_of(m)
                            src = s16a if m < 2 else s16b
                            nc.tensor.matmul(
                                ps[:, off:off + B], lhsT=ident16[:, :],
                                rhs=src[:, (m % 2) * B:(m % 2 + 1) * B],
                                start=(m % 2 == 0), stop=False,
                                skip_group_check=True)

                    def kmm(m, k, stop=False):
                        ps, off = ps_of(m)
                        return nc.tensor.matmul(
                            ps[:, off:off + B],
                            lhsT=w16[:, k * R + m * 128:k * R + (m + 1) * 128],
                            rhs=prev_r[:, k * B:(k + 1) * B],
                            start=False, stop=stop, skip_group_check=True)

                    # bank A injections + k01 (gated by r'a)
                    dinj(0); dinj(1)
                    kmm(0, 0); kmm(1, 0); kmm(0, 1); kmm(1, 1)
                    # k23m01 (gated by r'b) completes bank A asap
                    kmm(0, 2); kmm(1, 2); kmm(0, 3); kmm(1, 3, stop=True)
                    # bank C in the shadow: a-gated first, then b-gated
                    kmm(2, 0); kmm(3, 0); kmm(2, 1); kmm(3, 1)
                    dinj(2); dinj(3)
                    kmm(2, 2); kmm(3, 2); kmm(2, 3); kmm(3, 3, stop=True)

                # r' = relu(q * 0.2*0.8^(jp-jn))   (DVE, psum in only)
                s0 = float(ALPHA * LEAK ** (jp - jn))
                nc.vector._custom_dve(
                    RELU_SC,
                    out=rbig[:, :2 * B].rearrange("p (m c) -> p m c", c=B),
                    in0=pvA[:, 0:2, 0:B], s0=s0)
                nc.vector._custom_dve(
                    RELU_SC,
                    out=rbig[:, 2 * B:].rearrange("p (m c) -> p m c", c=B),
                    in0=pvC[:, 0:2, 0:B], s0=s0)
                # h output: h_t = 0.8*h_{t-1} + 0.8^jn * r'  (off critical path)
                nc.vector._custom_dve(
                    LEAK2,
                    out=h16[:, wr:wr + SUP], in0=h16[:, rd:rd + SUP],
                    in1=rbig[:, :], s0=float(LEAK), s1=float(LEAK ** jn))
                prev_r = rbig
                for ci, (a, e) in enumerate(ochunks):
                    if t == e:
                        emit_out_chunk(ci)
            for ci, (a, e) in enumerate(ochunks):
                if e >= T:
                    emit_out_chunk(ci)

    if use_bacc:
        nc.compile()
    return nc


def host_prep(x, w_in, w_rec, b_rec, ei_mask, autapse_mask, noise):
    """Host-side weight prep + time shard + pre-scaled fp16 delta inputs.

    delta_t = z_t - 0.8*z_{t-1} split into x and noise parts, scaled by
    0.8^-(t % Q) to match the psum accumulation frame. b_rec is folded
    into the noise part (constant offset of z).
    """
    ei = np.diagonal(np.asarray(ei_mask)).astype(np.float32)
    w_eff = ei[:, None] * (np.asarray(w_rec) * np.asarray(autapse_mask))
    w16 = w_eff.astype(np.float16)
    win16 = np.asarray(w_in).astype(np.float16)
    x = np.asarray(x, dtype=np.float32)
    nb = np.asarray(noise, dtype=np.float32) + np.asarray(b_rec, np.float32)
    jscale = (LEAK ** -(np.arange(T_LOC) % Q)).astype(np.float32)
    in_maps = []
    for c in range(N_CORES):
        t0 = c * T_OUT - BURN
        xp = np.zeros((B, T_LOC, NIN), np.float32)
        npad = np.zeros((B, T_LOC, R), np.float32)
        s = max(t0, 0)
        off = s - t0
        xp[:, off:] = x[:, s:t0 + T_LOC]
        npad[:, off:] = nb[:, s:t0 + T_LOC]
        dx = xp.copy()
        dx[:, 1:] -= LEAK * xp[:, :-1]
        dn = npad.copy()
        dn[:, 1:] -= LEAK * npad[:, :-1]
        dx *= jscale[None, :, None]
        dn *= jscale[None, :, None]
        dxT = np.ascontiguousarray(
            dx.astype(np.float16).transpose(2, 1, 0))
        dnT = np.ascontiguousarray(
            dn.astype(np.float16).reshape(B, T_LOC, RC, 128)
            .transpose(3, 2, 1, 0))
        in_maps.append({
            "dxT16": dxT,
            "dnT16": dnT,
            "w16": w16,
            "win16": win16,
        })
    return in_maps, w_eff.astype(np.float32)


def reference_np(x, w_in, b_rec, w_eff, noise, T=None):
    """Numpy reference for dev checks (f32)."""
    x = np.asarray(x, np.float32)
    if T is None:
        T = x.shape[1]
    z = np.einsum("bti,ir->btr", x[:, :T], np.asarray(w_in)) \
        + np.asarray(noise)[:, :T] + np.asarray(b_rec)
    h = np.zeros((x.shape[0], w_eff.shape[0]), np.float32)
    outs = []
    for t in range(T):
        pre = z[:, t] + h @ w_eff
        h = LEAK * h + ALPHA * np.maximum(pre, 0.0)
        outs.append(h.copy())
    return np.stack(outs, axis=1)


# ---------------------------------------------------------------------------
# harness entry point
# ---------------------------------------------------------------------------
_NC_CACHE = {}


def kernel(x, w_in, w_rec, b_rec, ei_mask, autapse_mask, noise):
    from concourse.bass_utils import run_bass_kernel_spmd

    x = np.asarray(x)
    T = x.shape[1]
    in_maps, _ = host_prep(x, w_in, w_rec, b_rec, ei_mask, autapse_mask, noise)
    if T not in _NC_CACHE:
        _NC_CACHE[T] = build_nc()
    nc = _NC_CACHE[T]
    res = run_bass_kernel_spmd(nc, in_maps, core_ids=list(range(N_CORES)))
    out = np.empty((x.shape[0], T, R), np.float32)
    for c in range(N_CORES):
        # dump[p, t, m*B+b] = h[b, t, m*128+p]
        dump = res.results[c]["outT16"]
        out[:, c * T_OUT:(c + 1) * T_OUT] = (
            dump.reshape(128, T_OUT, RC, B).transpose(3, 1, 2, 0)
            .reshape(B, T_OUT, R).astype(np.float32))
    return out
